# revision 50
# baseline (speedup 1.0000x reference)
"""AdaptiveGraphWaveletConv Trainium2 kernel (8 NeuronCores, SPMD).

Math (reference):
    mp(h)[d] = sum_{e: dst_e=d} w_e * h[src_e]          (per batch)
    T_0 = x; T_1 = mp(x); T_k = 2*mp(T_{k-1}) - T_{k-2} (K=3)
    out = sum_k T_k @ Theta0_k + s_local * (sum_k T_k @ Theta1_k) + bias

Strategy:
  - 8-way destination-node split (6250 nodes/core), all 4 batches fused into
    512 bf16 feature columns -> gather rows are 1KB (latency-optimal for the
    SWDGE dma_gather path, 4 queues).
  - Message passing per Chebyshev step: dma_gather h[src] rows from local HBM,
    TensorE scatter-reduce with host-precomputed weighted one-hot W^T blocks
    (lhsT=[128 edge slots, 128 local dst], rhs=gathered msgs [128, 512]),
    PSUM accumulation per 128-dst tile, VectorE Chebyshev update.
  - 8-rank AllGather replicates T_k to every core's HBM between steps.
  - Phase 2: DMA-transpose T_k slices, matmul against [Theta0|Theta1],
    bias via ones-row matmul, out0 + s*out1 on VectorE.

The per-(tile, src-half) slot counts are normalized to the max over all 8
cores so every core runs the identical instruction stream (SPMD), padding
with (idx=0, w=0) slots.
"""

import sys

sys.path.insert(0, "/opt/trn_rl_repo")

import os

import numpy as np
import ml_dtypes

from concourse import bass, bacc, mybir
from concourse.bass_utils import run_bass_kernel_spmd

last_exec_time_ns = None
last_trace_dir = None


def _maybe_install_ntff_hook():
    if not os.environ.get("BASS_KERNEL_TRACE"):
        return False
    import types
    import antenv
    if not hasattr(antenv, "axon_hooks"):
        _m = types.ModuleType("antenv.axon_hooks")
        _m._hook = None
        def set_axon_ntff_profile_hook(h): _m._hook = h
        def get_axon_ntff_profile_hook(): return _m._hook
        _m.set_axon_ntff_profile_hook = set_axon_ntff_profile_hook
        _m.get_axon_ntff_profile_hook = get_axon_ntff_profile_hook
        sys.modules["antenv.axon_hooks"] = _m
        antenv.axon_hooks = _m
        try:
            from trn_agent_boot.trn_boot import _ntff_profile_via_ctypes
            set_axon_ntff_profile_hook(
                _ntff_profile_via_ctypes("/opt/axon/libaxon_pjrt.so"))
        except Exception:
            return False
    return True

BF16 = mybir.dt.bfloat16
F32 = mybir.dt.float32
I16 = mybir.dt.int16

N_CORES = 8
NQ = 4  # SWDGE queues
LOW_CAP = 32768  # int16 index split (overridable for tests)
NCHUNK = 4  # AllGather chunks per step (overlap with step tail)
AG_SLACK = 5  # tiles of slack before waiting on a chunk's stores
DEBUG_DUMP = False  # add h1f/t3s debug outputs


# ---------------------------------------------------------------- host side


def _preprocess_edges(N, edge_index, edge_attr):
    """Edge-structure-dependent arrays (cacheable with the compiled graph)."""
    E = edge_index.shape[1]
    SLICE = N // N_CORES
    TILES = (SLICE + 127) // 128
    ROWPAD = TILES * 128
    PN = N_CORES * ROWPAD
    LOW = min(LOW_CAP, PN)

    dst = np.asarray(edge_index[0], dtype=np.int64)
    src = np.asarray(edge_index[1], dtype=np.int64)
    w = np.asarray(edge_attr, dtype=np.float32)

    # chunk-major global row layout for h buffers:
    #   [chunk j][core c][tile t-o_j][128 rows]
    # so each AllGather chunk writes one contiguous range of h_full.
    base_nt = TILES // NCHUNK
    cnt = np.full(NCHUNK, base_nt, dtype=np.int64)
    cnt[:TILES - base_nt * NCHUNK] += 1
    coff = np.zeros(NCHUNK, dtype=np.int64)
    np.cumsum(cnt[:-1], out=coff[1:])
    chunk_of_tile = np.repeat(np.arange(NCHUNK), cnt)
    cbase = np.zeros(NCHUNK, dtype=np.int64)
    np.cumsum((N_CORES * cnt * 128)[:-1], out=cbase[1:])

    def _psrow(node):
        c_s = node // SLICE
        r_s = node % SLICE
        t_s = r_s // 128
        d_s = r_s % 128
        j_s = chunk_of_tile[t_s]
        return cbase[j_s] + c_s * cnt[j_s] * 128 + (t_s - coff[j_s]) * 128 + d_s

    core = dst // SLICE
    tile = (dst % SLICE) // 128
    d_loc = (dst % SLICE) % 128
    ps = _psrow(src)
    half = (ps >= LOW).astype(np.int64)

    seg = tile * 2 + half
    seg_key = core * (TILES * 2) + seg
    counts = np.bincount(seg_key, minlength=N_CORES * TILES * 2) \
        .reshape(N_CORES, TILES * 2)
    sizes = counts.max(axis=0)
    sizes = ((sizes + 127) // 128) * 128
    # every tile needs >= 1 block so its PSUM group exists
    st = sizes.reshape(TILES, 2)
    st[st.sum(axis=1) == 0, 0] = 128
    sizes = st.reshape(-1)
    starts = np.zeros(TILES * 2 + 1, dtype=np.int64)
    np.cumsum(sizes, out=starts[1:])
    TOT = int(starts[-1])

    order = np.lexsort((ps, seg, core))
    core_s, seg_s = core[order], seg[order]
    dloc_s, ps_s, half_s, w_s = d_loc[order], ps[order], half[order], w[order]
    seg_key_s = core_s * (TILES * 2) + seg_s
    run_counts = np.bincount(seg_key_s, minlength=N_CORES * TILES * 2)
    run_starts = np.zeros(N_CORES * TILES * 2 + 1, dtype=np.int64)
    np.cumsum(run_counts, out=run_starts[1:])
    rank_in_run = np.arange(E) - run_starts[seg_key_s]
    slot = starts[seg_s] + rank_in_run

    IDX = np.full((N_CORES, TOT), -1, dtype=np.int16)
    WT = np.zeros((N_CORES, TOT, 128), dtype=ml_dtypes.bfloat16)
    IDX[core_s, slot] = (ps_s - half_s * LOW).astype(np.int16)
    WT[core_s, slot, dloc_s] = w_s.astype(ml_dtypes.bfloat16)
    # per-(core, seg) valid counts; per-call num_idxs_reg = max over cores
    # must equal THIS core's count -> but SPMD needs one immediate; the HW
    # contract only requires num_idxs_reg == count of non-negative for the
    # interp; on HW the register tells the Q7 how many to transfer. Using
    # the padded max means trailing -1s are "transferred"?? -- instead pad
    # each core's run to the call size with idx 0 beyond its own count is
    # wrong; so: make every core's valid count EQUAL by padding shorter
    # cores with repeats of index 0 up to the max count, then -1 to the
    # call boundary.
    cnt_cs = counts  # [N_CORES, TILES*2]
    for t2 in range(TILES * 2):
        mx = int(cnt_cs[:, t2].max())
        if mx == 0 and sizes[t2] > 0:
            mx = 1
        s0v = int(starts[t2])
        for c2 in range(N_CORES):
            k2 = int(cnt_cs[c2, t2])
            if k2 < mx:
                IDX[c2, s0v + k2:s0v + mx] = 0
    valid = np.zeros(TILES * 2, dtype=np.int64)
    for t2 in range(TILES * 2):
        mx = int(cnt_cs[:, t2].max())
        if mx == 0 and sizes[t2] > 0:
            mx = 1
        valid[t2] = mx

    nL = (sizes.reshape(TILES, 2)[:, 0] // 128).tolist()
    nH = (sizes.reshape(TILES, 2)[:, 1] // 128).tolist()
    MAXBLK = int(max(nL[t] + nH[t] for t in range(TILES)))

    # gather-call list + wrapped idx tensor. One call per (tile, half);
    # the queue is assigned at build time as parity*2 + half so that at
    # most ONE triggered call is ever outstanding per queue (the 16
    # per-engine sem increments of two in-flight calls on one queue would
    # otherwise be indistinguishable -> torn-read race on msgs).
    idx_cols = TOT // 16
    IDXW = np.zeros((N_CORES, 128, idx_cols), dtype=np.int16)
    colp = 0
    call_list = []  # (tile, half, n_slots, idx_col_off, blk_off, n_valid)
    for t in range(TILES):
        blk_off = 0
        for h in (0, 1):
            n = int(sizes[t * 2 + h])
            if n == 0:
                continue
            s0 = int(starts[t * 2 + h])
            seg_idx = IDX[:, s0:s0 + n]
            IDXW[:, 0:16, colp:colp + n // 16] = (
                seg_idx.reshape(N_CORES, n // 16, 16).transpose(0, 2, 1))
            nv = int(valid[t * 2 + h])
            call_list.append((t, h, n, colp, blk_off, nv))
            colp += n // 16
            blk_off += n // 128
    IDXW[:, 16:128, :] = np.tile(IDXW[:, 0:16, :], (1, 7, 1))
    assert colp == idx_cols

    WT_pm = np.ascontiguousarray(
        WT.reshape(N_CORES, TOT // 128, 128, 128).transpose(0, 2, 1, 3)
        .reshape(N_CORES, 128, (TOT // 128) * 128))

    node_ps = _psrow(np.arange(N, dtype=np.int64))
    cfg = dict(N=N, E=E, SLICE=SLICE, TILES=TILES, ROWPAD=ROWPAD, PN=PN,
               LOW=LOW, TOT=TOT, MAXBLK=MAXBLK, call_list=call_list,
               nL=nL, nH=nH, chunk_cnt=cnt, chunk_off=coff, chunk_base=cbase,
               node_ps=node_ps)
    return cfg, IDXW, WT_pm


def _preprocess_values(cfg, x, s_local):
    """x / s_local dependent arrays (recomputed every call)."""
    B, N, F = x.shape
    COLS = B * F
    SLICE, TILES, ROWPAD, PN = cfg["SLICE"], cfg["TILES"], cfg["ROWPAD"], cfg["PN"]

    xb = np.ascontiguousarray(np.asarray(x, np.float32).transpose(1, 0, 2)
                              .reshape(N, COLS)).astype(ml_dtypes.bfloat16)
    h0 = np.zeros((PN, COLS), dtype=ml_dtypes.bfloat16)
    h0[cfg["node_ps"]] = xb
    xs = np.zeros((N_CORES * ROWPAD, COLS), dtype=ml_dtypes.bfloat16)
    for c in range(N_CORES):
        xs[c * ROWPAD:c * ROWPAD + SLICE] = xb[c * SLICE:(c + 1) * SLICE]
    x_slice = np.ascontiguousarray(xs.reshape(N_CORES, ROWPAD, COLS))
    x_slice_pm = np.ascontiguousarray(
        x_slice.reshape(N_CORES, TILES, 128, COLS).transpose(0, 2, 1, 3)
        .reshape(N_CORES, 128, TILES * COLS))

    s_pm = np.zeros((N_CORES, 128, TILES * B), dtype=np.float32)
    s_t = np.asarray(s_local, dtype=np.float32)
    for c in range(N_CORES):
        sl = np.zeros((ROWPAD, B), dtype=np.float32)
        sl[:SLICE] = s_t[:, c * SLICE:(c + 1) * SLICE].T
        s_pm[c] = sl.reshape(TILES, 128, B).transpose(1, 0, 2).reshape(128, TILES * B)
    return dict(h0=h0, x_slice=x_slice, x_slice_pm=x_slice_pm, s_pm=s_pm,
                B=B, F=F, COLS=COLS)


# ---------------------------------------------------------------- bass build


def _build(cfg, B, F, K1):
    COLS = B * F
    TILES, ROWPAD, PN = cfg["TILES"], cfg["ROWPAD"], cfg["PN"]
    LOW, TOT, MAXBLK = cfg["LOW"], cfg["TOT"], cfg["MAXBLK"]
    call_list = cfg["call_list"]
    nL, nH = cfg["nL"], cfg["nH"]
    c_cnt = [int(v) for v in cfg["chunk_cnt"]]
    c_off = [int(v) for v in cfg["chunk_off"]]
    c_base = [int(v) for v in cfg["chunk_base"]]
    chunk_of_tile = [j for j in range(NCHUNK) for _ in range(c_cnt[j])]
    # AG chunk j fires after the gathers of this tile are issued
    fire_after_tile = {t: [] for t in range(TILES)}
    for j in range(NCHUNK):
        ft = min(c_off[j] + c_cnt[j] - 1 + AG_SLACK, TILES - 1)
        fire_after_tile[ft].append(j)

    nc = bacc.Bacc("TRN2", debug=False, num_swdge_queues=NQ,
                   dynamic_dma_scratch_size=32768)

    h0_ext = nc.declare_dram_parameter("h0", [PN, COLS], BF16, isOutput=False)
    idx_ext = nc.declare_dram_parameter("idxw", [128, TOT // 16], I16, isOutput=False)
    wt_ext = nc.declare_dram_parameter("wt", [128, (TOT // 128) * 128], BF16, isOutput=False)
    xs_ext = nc.declare_dram_parameter("x_slice", [ROWPAD, COLS], BF16, isOutput=False)
    xspm_ext = nc.declare_dram_parameter("x_slice_pm", [128, TILES * COLS], BF16, isOutput=False)
    s_ext = nc.declare_dram_parameter("s_pm", [128, TILES * B], F32, isOutput=False)
    th_ext = nc.declare_dram_parameter("theta", [K1 * F, 2 * F], BF16, isOutput=False)
    bias_ext = nc.declare_dram_parameter("bias2", [1, 2 * F], BF16, isOutput=False)
    # partition-major output: row = bi*128 + d, col = t*F + f, so one store
    # covers 4 tiles contiguously per partition (fewer DMA descriptors)
    out_ext = nc.declare_dram_parameter("out", [B * 128, TILES * F], F32, isOutput=True)
    GRPB = (TILES + 3) // 4  # out-store groups per batch
    if DEBUG_DUMP:
        dbg1_ext = nc.declare_dram_parameter("dbg1", [PN, COLS], BF16, isOutput=True)
        dbg3_ext = nc.declare_dram_parameter("dbg3", [ROWPAD, COLS], BF16, isOutput=True)
        dbg0_ext = nc.declare_dram_parameter("dbg0", [ROWPAD, COLS], BF16, isOutput=True)

    t_sl = [None,
            nc.dram_tensor("t1s", [ROWPAD, COLS], BF16),
            nc.dram_tensor("t2s", [ROWPAD, COLS], BF16),
            nc.dram_tensor("t3s", [ROWPAD, COLS], BF16)]
    h_full = [None,
              nc.dram_tensor("h1f", [PN, COLS], BF16, addr_space="Shared"),
              nc.dram_tensor("h2f", [PN, COLS], BF16, addr_space="Shared")]
    warm_in = nc.dram_tensor("warm_in", [128, 64], BF16)
    warm_out = nc.dram_tensor("warm_out", [N_CORES * 128, 64], BF16,
                              addr_space="Shared")
    groups = [list(range(N_CORES))]

    calls_per_tile = {t: [] for t in range(TILES)}
    for (t, h, n, coff, boff, nv) in call_list:
        calls_per_tile[t].append((h, n, coff, boff, nv))

    tile_blk0 = []
    acc = 0
    for t in range(TILES):
        tile_blk0.append(acc)
        acc += nL[t] + nH[t]
    tile_nblk = [nL[t] + nH[t] for t in range(TILES)]

    # cumulative per-queue gather-sem / W-sem targets per (step, tile).
    # Queue = parity*2 + half; every call on queue q bumps gq[q] by 16 at
    # DMA completion, and at most one triggered call is in flight per queue.
    gtgt = {}
    wtgt = {}
    _g = [0, 0, 0, 0]
    _w = [0, 0]
    for step in range(3):
        for t in range(TILES):
            b = (step * TILES + t) % 2
            for (h, n, coff, boff, nv) in calls_per_tile[t]:
                _g[b * 2 + h] += 16
            gtgt[(step, t)] = tuple(_g)
            _w[b] += 16
            wtgt[(step, t)] = _w[b]

    from contextlib import ExitStack
    _es = ExitStack()
    with _es:
        sem = lambda n: _es.enter_context(nc.semaphore(n))
        sbuf = lambda n, s, d: _es.enter_context(nc.sbuf_tensor(n, s, d))
        idxS = sem("idxS"); xpmS = sem("xpmS")
        gq = [sem(f"gq{i}") for i in range(4)]
        pq = [sem(f"pq{i}") for i in range(4)]
        wSA = sem("wSA"); wSB = sem("wSB"); mmS = sem("mmS"); mmL = sem("mmL")
        evS = sem("evS"); ccS = sem("ccS"); onesS = sem("onesS")
        stC = [sem(f"stC{i}") for i in range(NCHUNK)]
        msS = sem("msS"); p2L = sem("p2L"); p2mm = sem("p2mm"); p2ev = sem("p2ev"); p2cp = sem("p2cp")
        p2T = [sem("p2T0"), sem("p2T1")]
        p2st = [sem(f"p2st{i}") for i in range(4)]
        msgsA = sbuf("msgsA", [128, MAXBLK, COLS], BF16)
        msgsB = sbuf("msgsB", [128, MAXBLK, COLS], BF16)
        wbufA = sbuf("wbufA", [128, MAXBLK, 128], BF16)
        wbufB = sbuf("wbufB", [128, MAXBLK, 128], BF16)
        idxs = sbuf("idxs", [128, TOT // 16], I16)
        arena0 = sbuf("arena0", [128, TILES * COLS], BF16)
        arena1 = sbuf("arena1", [128, TILES * COLS], BF16)
        ssb = sbuf("ssb", [128, TILES * B], F32)
        thsb = sbuf("thsb", [128, K1, 2 * F], BF16)
        ones1 = sbuf("ones1", [1, 128], BF16)
        bias_sb = sbuf("bias_sb", [1, 2 * F], BF16)
        outsb = [sbuf(f"outsb{i}", [128, 4 * F], F32) for i in range(2)]
        psA = _es.enter_context(nc.psum_tensor("psA", [128, COLS], F32))
        psB = _es.enter_context(nc.psum_tensor("psB", [128, COLS], F32))
        ps2 = [_es.enter_context(nc.psum_tensor(f"ps2{i}", [128, 2 * F], F32))
               for i in range(4)]
        msgs = [msgsA, msgsB]
        wbuf = [wbufA, wbufB]
        psum = [psA, psB]
        arenas = [arena0, arena1]
        # arena roles: arena0 = x -> T2 (in place at step 1) -> Tt[0:2]
        #              arena1 = T1 -> T3 (in place at step 2? no: cur list) -> Tt[2:4]
        prev_arena = [None, arena0, arena1]
        cur_arena = [arena1, arena0, arena1]   # steps 1,2 update in place
        wsem = [wSA, wSB]
        step_src = [h0_ext, h_full[1], h_full[2]]

        # which step stored which arena most recently, for overwrite waits:
        # cur_arena[1] = arena0 written at step1 (stores at step1);
        # cur_arena[2] = arena0 again -> step2 evac tile t must wait step1's
        # store of tile t: stS >= 16*(1*TILES + t + 1).

        # ---------------- phase 1
        with nc.Block() as blk:

            @blk.gpsimd
            def _(gpsimd):
                gpsimd.dma_start(out=idxs[:, :], in_=idx_ext[:, :]).then_inc(idxS, 16)
                gpsimd.dma_start(out=arena0[:, :], in_=xspm_ext[:, :]).then_inc(xpmS, 16)
                # dummy collective: absorbs ncfw first-collective warm-up
                # while the initial loads run (nothing waits on it; later
                # collectives queue behind it in straight-line order)
                gpsimd.collective_compute(
                    "AllGather",
                    mybir.AluOpType.bypass,
                    replica_groups=groups,
                    ins=[warm_in.ap().opt()],
                    outs=[warm_out.ap().opt()],
                ).then_inc(ccS)
                gpsimd.wait_ge(msS, 2)
                gpsimd.wait_ge(idxS, 16)

                NG = 3 * TILES
                prep_cnt = [0, 0, 0, 0]          # preps issued per queue
                trig_cnt = [0, 0, 0, 0]          # triggers issued per queue
                prep_tgt = {}                    # (g, q) -> prep_cnt to wait for

                def prep_tile(gp):
                    stepp, tp = gp // TILES, gp % TILES
                    bp = gp % 2
                    src_t = step_src[stepp]
                    for (h, n, coff, boff, nv) in calls_per_tile[tp]:
                        q = bp * 2 + h
                        src_ap = src_t[0:LOW, :] if h == 0 else src_t[LOW:PN, :]
                        gpsimd.dma_gather(
                            msgs[bp][:, boff:boff + n // 128, :],
                            src_ap,
                            idxs[:, coff:coff + n // 16],
                            n, nv, COLS,
                            single_packet=False,
                            queue_num=q,
                            prepare_only=True,
                            sem=gq[q],
                        ).then_inc(pq[q], 1)
                        prep_cnt[q] += 1
                        prep_tgt[(gp, q)] = prep_cnt[q]

                PRE = 2
                for gp in range(min(PRE, NG)):
                    prep_tile(gp)
                g = 0
                for step in range(3):
                    if step > 0:
                        # L calls read src rows [0, LOW) only -- covered by
                        # AG chunks 0..2 (chunk 3 starts above LOW). Fire the
                        # first two tiles' L drains under the final chunk's
                        # AllGather; H calls wait for the full gather source.
                        # (ccS: +1 for the warm-up collective; chunks done
                        # in issue order since collectives serialize.)
                        gpsimd.wait_ge(ccS, step * NCHUNK)
                        for tt in (0, 1):
                            gg = step * TILES + tt
                            bb = gg % 2
                            # msgs[bb] reuse: matmul gg-2 (prev step) done
                            gpsimd.wait_ge(mmS, gg - 1)
                            for (h, n, coff, boff, nv) in calls_per_tile[tt]:
                                if h == 0:
                                    q = bb * 2
                                    gpsimd.wait_ge(pq[q], prep_tgt[(gg, q)])
                                    gpsimd.trigger_dma(count=1, queue_num=q)
                                    trig_cnt[q] += 1
                        gpsimd.wait_ge(ccS, step * NCHUNK + 1)
                        for tt in (0, 1):
                            gg = step * TILES + tt
                            bb = gg % 2
                            for (h, n, coff, boff, nv) in calls_per_tile[tt]:
                                if h == 1:
                                    q = bb * 2 + 1
                                    gpsimd.wait_ge(pq[q], prep_tgt[(gg, q)])
                                    gpsimd.trigger_dma(count=1, queue_num=q)
                                    trig_cnt[q] += 1
                    for t in range(TILES):
                        b = g % 2
                        pretrig = step > 0 and t < 2
                        for (h, n, coff, boff, nv) in calls_per_tile[t]:
                            if pretrig:
                                continue
                            q = b * 2 + h
                            if g >= 2:
                                # L region frees once matmul g-2's L blocks
                                # retire (mmL) -- but only if this tile's L
                                # region does not spill into g-2's H blocks
                                # (nL varies per tile). H and spilling L need
                                # the full group (mmS).
                                tp2 = (g - 2) % TILES
                                early = h == 0 and nL[t] <= nL[tp2]
                                gpsimd.wait_ge(mmL if early else mmS, g - 1)
                            gpsimd.wait_ge(pq[q], prep_tgt[(g, q)])
                            gpsimd.trigger_dma(count=1, queue_num=q)
                            trig_cnt[q] += 1
                        g += 1
                        if g + PRE - 1 < NG:
                            prep_tile(g + PRE - 1)
                        if step < 2:
                            for j in fire_after_tile[t]:
                                r0 = c_off[j] * 128
                                r1 = (c_off[j] + c_cnt[j]) * 128
                                gpsimd.wait_ge(
                                    stC[j], 16 * c_cnt[j] * (step + 1))
                                gpsimd.collective_compute(
                                    "AllGather",
                                    mybir.AluOpType.bypass,
                                    replica_groups=groups,
                                    ins=[t_sl[step + 1][r0:r1, :].opt()],
                                    outs=[h_full[step + 1][
                                        c_base[j]:c_base[j] + N_CORES * (r1 - r0),
                                        :].opt()],
                                ).then_inc(ccS)

            @blk.sync
            def _(sync):
                for step in range(3):
                    for t in range(TILES):
                        g = step * TILES + t
                        b = g % 2
                        if g >= 2:
                            sync.wait_ge(mmS, g - 1)
                        nb = tile_nblk[t]
                        blk0 = tile_blk0[t]
                        sync.dma_start(
                            out=wbuf[b][:, 0:nb, :],
                            in_=wt_ext[:, blk0 * 128:(blk0 + nb) * 128],
                        ).then_inc(wsem[b], 16)

            @blk.tensor
            def _(tensor):
                prev_tgt = [0, 0, 0, 0]
                for step in range(3):
                    for t in range(TILES):
                        g = step * TILES + t
                        b = g % 2
                        if g >= 2:
                            # psum bank reuse: vector must have evacuated g-2
                            tensor.wait_ge(evS, g - 1)
                        tgts = gtgt[(step, t)]
                        tensor.wait_ge(wsem[b], wtgt[(step, t)])
                        nb = tile_nblk[t]
                        nbL = nL[t]
                        ins = None
                        # L blocks as soon as the L gather lands; H blocks
                        # (usually still draining) waited on separately.
                        if tgts[b * 2] > prev_tgt[b * 2]:
                            tensor.wait_ge(gq[b * 2], tgts[b * 2])
                        for blkno in range(nbL):
                            ins = tensor.matmul(
                                psum[b][:, :],
                                wbuf[b][:, blkno, :],
                                msgs[b][:, blkno, :],
                                start=(blkno == 0),
                                stop=(blkno == nb - 1),
                            )
                        ins.then_inc(mmL, 1)
                        if nb > nbL:
                            if tgts[b * 2 + 1] > prev_tgt[b * 2 + 1]:
                                tensor.wait_ge(gq[b * 2 + 1], tgts[b * 2 + 1])
                            for blkno in range(nbL, nb):
                                ins = tensor.matmul(
                                    psum[b][:, :],
                                    wbuf[b][:, blkno, :],
                                    msgs[b][:, blkno, :],
                                    start=False,
                                    stop=(blkno == nb - 1),
                                )
                        prev_tgt = list(tgts)
                        ins.then_inc(mmS, 1)

            @blk.vector
            def _(vector):
                vector.memset(ones1[:, :], 1.0).then_inc(onesS, 1)
                vector.memset(msgsA[:, :, :], 0.0).then_inc(msS, 1)
                vector.memset(msgsB[:, :, :], 0.0).then_inc(msS, 1)
                for step in range(3):
                    prev = prev_arena[step]
                    cur = cur_arena[step]
                    if step == 1:
                        vector.wait_ge(xpmS, 16)
                    for t in range(TILES):
                        b = (step * TILES + t) % 2
                        vector.wait_ge(mmS, step * TILES + t + 1)
                        dst = cur[:, t * COLS:(t + 1) * COLS]
                        if step == 0:
                            vector.tensor_scalar_mul(dst, psum[b][:, :], 1.0) \
                                .then_inc(evS, 1)
                        else:
                            vector.scalar_tensor_tensor(
                                dst,
                                psum[b][:, :],
                                2.0,
                                prev[:, t * COLS:(t + 1) * COLS],
                                op0=mybir.AluOpType.mult,
                                op1=mybir.AluOpType.subtract,
                            ).then_inc(evS, 1)

            @blk.scalar
            def _(scalar):
                for step in range(3):
                    cur = cur_arena[step]
                    for t in range(TILES):
                        scalar.wait_ge(evS, step * TILES + t + 1)
                        scalar.dma_start(
                            out=t_sl[step + 1][t * 128:(t + 1) * 128, :],
                            in_=cur[:, t * COLS:(t + 1) * COLS],
                        ).then_inc(stC[chunk_of_tile[t]], 16)

        # ---------------- phase 2
        with nc.Block() as blk2:

            @blk2.sync
            def _(sync):
                # theta/s/bias loads do not depend on phase-1 state
                sync.dma_start(out=ssb[:, :], in_=s_ext[:, :]).then_inc(p2L, 16)
                sync.dma_start(
                    out=thsb[:, :, :],
                    in_=th_ext[:, :].rearrange("(k f) o -> f k o", k=K1),
                ).then_inc(p2L, 16)
                sync.dma_start(out=bias_sb[:, :], in_=bias_ext[:, :]).then_inc(p2L, 16)
                for j in range(NCHUNK):
                    sync.wait_ge(stC[j], 16 * c_cnt[j] * 3)
                srcs = [xs_ext, t_sl[1], t_sl[2], t_sl[3]]
                # batch-parity double-buffered staging: arenas[bi%2] holds
                # the 4 transposed k-slices of batch bi (exactly 4*ROWPAD
                # cols), so batch bi+1 transposes overlap batch bi matmuls.
                for bi in range(B):
                    if bi >= 2:
                        sync.wait_ge(p2mm, (bi - 1) * TILES)
                    for k in range(K1):
                        dst = arenas[bi % 2][:, k * ROWPAD:k * ROWPAD + ROWPAD]
                        sync.dma_start_transpose(
                            dst, srcs[k][:, bi * F:(bi + 1) * F],
                        ).then_inc(p2T[bi % 2], 16)

            @blk2.tensor
            def _(tensor):
                tensor.wait_ge(onesS, 1)
                tensor.wait_ge(p2L, 48)
                for bi in range(B):
                    tensor.wait_ge(p2T[bi % 2], 16 * K1 * (bi // 2 + 1))
                    for t in range(TILES):
                        i = bi * TILES + t
                        pb = i % 4
                        if i >= 4:
                            tensor.wait_ge(p2ev, i - 3)
                        for k in range(K1):
                            src = arenas[bi % 2][:, k * ROWPAD + t * 128:
                                                 k * ROWPAD + (t + 1) * 128]
                            tensor.matmul(
                                ps2[pb][:, :],
                                src,
                                thsb[:, k, :],
                                start=(k == 0),
                                stop=False,
                            )
                        tensor.matmul(
                            ps2[pb][:, :],
                            ones1[:, :],
                            bias_sb[:, :],
                            start=False,
                            stop=True,
                        ).then_inc(p2mm, 1)

            @blk2.vector
            def _(vector):
                for bi in range(B):
                    for t in range(TILES):
                        i = bi * TILES + t
                        pb = i % 4
                        grp = bi * GRPB + t // 4
                        sl = t % 4
                        vector.wait_ge(p2mm, i + 1)
                        if sl == 0 and grp >= 2:
                            vector.wait_ge(p2st[grp % 2], 16 * (grp // 2))
                        dst = outsb[grp % 2][:, sl * F:(sl + 1) * F]
                        vector.tensor_scalar_mul(
                            dst, ps2[pb][:, 0:F], 1.0) \
                            .then_inc(p2cp, 1)
                        vector.wait_ge(p2cp, i + 1)
                        vector.scalar_tensor_tensor(
                            dst,
                            ps2[pb][:, F:2 * F],
                            ssb[:, (t * B + bi):(t * B + bi) + 1],
                            dst,
                            op0=mybir.AluOpType.mult,
                            op1=mybir.AluOpType.add,
                        ).then_inc(p2ev, 1)

            @blk2.scalar
            def _(scalar):
                for bi in range(B):
                    for t0 in range(0, TILES, 4):
                        gsz = min(4, TILES - t0)
                        grp = bi * GRPB + t0 // 4
                        i_last = bi * TILES + t0 + gsz - 1
                        scalar.wait_ge(p2ev, i_last + 1)
                        scalar.dma_start(
                            out=out_ext[bi * 128:(bi + 1) * 128,
                                        t0 * F:(t0 + gsz) * F],
                            in_=outsb[grp % 2][:, 0:gsz * F],
                        ).then_inc(p2st[grp % 2], 16)

        if DEBUG_DUMP:
            with nc.Block() as blk3:
                @blk3.sync
                def _(sync):
                    for _q in range(4):
                        sync.wait_ge(p2st[_q], 16)
                    base = 16
                    for i in range(PN // 128):
                        buf = msgs[i % 2][:, 0, :]
                        sync.dma_start(out=buf, in_=h_full[1][i * 128:(i + 1) * 128, :]).then_inc(idxS, 16)
                        sync.wait_ge(idxS, base + 16)
                        sync.dma_start(out=dbg1_ext[i * 128:(i + 1) * 128, :], in_=buf).then_inc(idxS, 16)
                        sync.wait_ge(idxS, base + 32)
                        base += 32
                    for i in range(ROWPAD // 128):
                        buf = msgs[i % 2][:, 0, :]
                        sync.dma_start(out=buf, in_=t_sl[3][i * 128:(i + 1) * 128, :]).then_inc(idxS, 16)
                        sync.wait_ge(idxS, base + 16)
                        sync.dma_start(out=dbg3_ext[i * 128:(i + 1) * 128, :], in_=buf).then_inc(idxS, 16)
                        sync.wait_ge(idxS, base + 32)
                        base += 32
                    for i in range(ROWPAD // 128):
                        buf = msgs[i % 2][:, 0, :]
                        sync.dma_start(out=buf, in_=t_sl[1][i * 128:(i + 1) * 128, :]).then_inc(idxS, 16)
                        sync.wait_ge(idxS, base + 16)
                        sync.dma_start(out=dbg0_ext[i * 128:(i + 1) * 128, :], in_=buf).then_inc(idxS, 16)
                        sync.wait_ge(idxS, base + 32)
                        base += 32

    nc.finalize()
    return nc


# ---------------------------------------------------------------- entry

_cache = {}


def _get_graph(N, B, F, K1, edge_index, edge_attr):
    key = (N, B, F, K1,
           hash(np.asarray(edge_index).tobytes()),
           hash(np.asarray(edge_attr).tobytes()))
    if key in _cache:
        return _cache[key]
    cfg, IDXW, WT_pm = _preprocess_edges(N, edge_index, edge_attr)
    nc = _build(cfg, B, F, K1)
    _cache.clear()
    _cache[key] = (cfg, IDXW, WT_pm, nc)
    return _cache[key]


def kernel(x, edge_index, edge_attr, s_local, Theta0, Theta1, bias):
    x = np.asarray(x)
    B, N, F = x.shape
    K1 = np.asarray(Theta0).shape[0]
    cfg, IDXW, WT_pm, nc = _get_graph(N, B, F, K1, edge_index, edge_attr)
    vals = _preprocess_values(cfg, x, s_local)
    SLICE, ROWPAD = cfg["SLICE"], cfg["ROWPAD"]
    COLS = vals["COLS"]

    th = np.concatenate([np.asarray(Theta0, np.float32),
                         np.asarray(Theta1, np.float32)], axis=2)
    th_b = np.ascontiguousarray(th).astype(ml_dtypes.bfloat16).reshape(K1 * F, 2 * F)
    bias2 = np.zeros((1, 2 * F), dtype=ml_dtypes.bfloat16)
    bias2[0, :F] = np.asarray(bias, np.float32).astype(ml_dtypes.bfloat16)

    in_maps = []
    for c in range(N_CORES):
        in_maps.append({
            "h0": vals["h0"],
            "idxw": np.ascontiguousarray(IDXW[c]),
            "wt": np.ascontiguousarray(WT_pm[c]),
            "x_slice": np.ascontiguousarray(vals["x_slice"][c]),
            "x_slice_pm": np.ascontiguousarray(vals["x_slice_pm"][c]),
            "s_pm": np.ascontiguousarray(vals["s_pm"][c]),
            "theta": th_b,
            "bias2": bias2,
        })

    trace = _maybe_install_ntff_hook()
    import tempfile
    tdir = tempfile.mkdtemp() if trace else None
    res = run_bass_kernel_spmd(nc, in_maps, core_ids=list(range(N_CORES)),
                               trace=trace, tmpdir=tdir)
    global last_exec_time_ns, last_trace_dir
    last_exec_time_ns = res.exec_time_ns
    last_trace_dir = tdir
    TILES = cfg["TILES"]
    out = np.empty((B, N, F), dtype=np.float32)
    for c in range(N_CORES):
        oc = res.results[c]["out"]  # [B*128, TILES*F] partition-major
        ocr = oc.reshape(B, 128, TILES, F).transpose(0, 2, 1, 3) \
                .reshape(B, ROWPAD, F)
        out[:, c * SLICE:(c + 1) * SLICE, :] = ocr[:, :SLICE]
    return out



# revision 51
# speedup vs baseline: 1.0059x; 1.0059x over previous
"""AdaptiveGraphWaveletConv Trainium2 kernel (8 NeuronCores, SPMD).

Math (reference):
    mp(h)[d] = sum_{e: dst_e=d} w_e * h[src_e]          (per batch)
    T_0 = x; T_1 = mp(x); T_k = 2*mp(T_{k-1}) - T_{k-2} (K=3)
    out = sum_k T_k @ Theta0_k + s_local * (sum_k T_k @ Theta1_k) + bias

Strategy:
  - 8-way destination-node split (6250 nodes/core), all 4 batches fused into
    512 bf16 feature columns -> gather rows are 1KB (latency-optimal for the
    SWDGE dma_gather path, 4 queues).
  - Message passing per Chebyshev step: dma_gather h[src] rows from local HBM,
    TensorE scatter-reduce with host-precomputed weighted one-hot W^T blocks
    (lhsT=[128 edge slots, 128 local dst], rhs=gathered msgs [128, 512]),
    PSUM accumulation per 128-dst tile, VectorE Chebyshev update.
  - 8-rank AllGather replicates T_k to every core's HBM between steps.
  - Phase 2: DMA-transpose T_k slices, matmul against [Theta0|Theta1],
    bias via ones-row matmul, out0 + s*out1 on VectorE.

The per-(tile, src-half) slot counts are normalized to the max over all 8
cores so every core runs the identical instruction stream (SPMD), padding
with (idx=0, w=0) slots.
"""

import sys

sys.path.insert(0, "/opt/trn_rl_repo")

import os

import numpy as np
import ml_dtypes

from concourse import bass, bacc, mybir
from concourse.bass_utils import run_bass_kernel_spmd

last_exec_time_ns = None
last_trace_dir = None


def _maybe_install_ntff_hook():
    if not os.environ.get("BASS_KERNEL_TRACE"):
        return False
    import types
    import antenv
    if not hasattr(antenv, "axon_hooks"):
        _m = types.ModuleType("antenv.axon_hooks")
        _m._hook = None
        def set_axon_ntff_profile_hook(h): _m._hook = h
        def get_axon_ntff_profile_hook(): return _m._hook
        _m.set_axon_ntff_profile_hook = set_axon_ntff_profile_hook
        _m.get_axon_ntff_profile_hook = get_axon_ntff_profile_hook
        sys.modules["antenv.axon_hooks"] = _m
        antenv.axon_hooks = _m
        try:
            from trn_agent_boot.trn_boot import _ntff_profile_via_ctypes
            set_axon_ntff_profile_hook(
                _ntff_profile_via_ctypes("/opt/axon/libaxon_pjrt.so"))
        except Exception:
            return False
    return True

BF16 = mybir.dt.bfloat16
F32 = mybir.dt.float32
I16 = mybir.dt.int16

N_CORES = 8
NQ = 4  # SWDGE queues
LOW_CAP = 32768  # int16 index split (overridable for tests)
NCHUNK = 4  # AllGather chunks per step (overlap with step tail)
AG_SLACK = 4  # tiles of slack before waiting on a chunk's stores
DEBUG_DUMP = False  # add h1f/t3s debug outputs


# ---------------------------------------------------------------- host side


def _preprocess_edges(N, edge_index, edge_attr):
    """Edge-structure-dependent arrays (cacheable with the compiled graph)."""
    E = edge_index.shape[1]
    SLICE = N // N_CORES
    TILES = (SLICE + 127) // 128
    ROWPAD = TILES * 128
    PN = N_CORES * ROWPAD
    LOW = min(LOW_CAP, PN)

    dst = np.asarray(edge_index[0], dtype=np.int64)
    src = np.asarray(edge_index[1], dtype=np.int64)
    w = np.asarray(edge_attr, dtype=np.float32)

    # chunk-major global row layout for h buffers:
    #   [chunk j][core c][tile t-o_j][128 rows]
    # so each AllGather chunk writes one contiguous range of h_full.
    base_nt = TILES // NCHUNK
    cnt = np.full(NCHUNK, base_nt, dtype=np.int64)
    cnt[:TILES - base_nt * NCHUNK] += 1
    coff = np.zeros(NCHUNK, dtype=np.int64)
    np.cumsum(cnt[:-1], out=coff[1:])
    chunk_of_tile = np.repeat(np.arange(NCHUNK), cnt)
    cbase = np.zeros(NCHUNK, dtype=np.int64)
    np.cumsum((N_CORES * cnt * 128)[:-1], out=cbase[1:])

    def _psrow(node):
        c_s = node // SLICE
        r_s = node % SLICE
        t_s = r_s // 128
        d_s = r_s % 128
        j_s = chunk_of_tile[t_s]
        return cbase[j_s] + c_s * cnt[j_s] * 128 + (t_s - coff[j_s]) * 128 + d_s

    core = dst // SLICE
    tile = (dst % SLICE) // 128
    d_loc = (dst % SLICE) % 128
    ps = _psrow(src)
    half = (ps >= LOW).astype(np.int64)

    seg = tile * 2 + half
    seg_key = core * (TILES * 2) + seg
    counts = np.bincount(seg_key, minlength=N_CORES * TILES * 2) \
        .reshape(N_CORES, TILES * 2)
    sizes = counts.max(axis=0)
    sizes = ((sizes + 127) // 128) * 128
    # every tile needs >= 1 block so its PSUM group exists
    st = sizes.reshape(TILES, 2)
    st[st.sum(axis=1) == 0, 0] = 128
    sizes = st.reshape(-1)
    starts = np.zeros(TILES * 2 + 1, dtype=np.int64)
    np.cumsum(sizes, out=starts[1:])
    TOT = int(starts[-1])

    order = np.lexsort((ps, seg, core))
    core_s, seg_s = core[order], seg[order]
    dloc_s, ps_s, half_s, w_s = d_loc[order], ps[order], half[order], w[order]
    seg_key_s = core_s * (TILES * 2) + seg_s
    run_counts = np.bincount(seg_key_s, minlength=N_CORES * TILES * 2)
    run_starts = np.zeros(N_CORES * TILES * 2 + 1, dtype=np.int64)
    np.cumsum(run_counts, out=run_starts[1:])
    rank_in_run = np.arange(E) - run_starts[seg_key_s]
    slot = starts[seg_s] + rank_in_run

    IDX = np.full((N_CORES, TOT), -1, dtype=np.int16)
    WT = np.zeros((N_CORES, TOT, 128), dtype=ml_dtypes.bfloat16)
    IDX[core_s, slot] = (ps_s - half_s * LOW).astype(np.int16)
    WT[core_s, slot, dloc_s] = w_s.astype(ml_dtypes.bfloat16)
    # per-(core, seg) valid counts; per-call num_idxs_reg = max over cores
    # must equal THIS core's count -> but SPMD needs one immediate; the HW
    # contract only requires num_idxs_reg == count of non-negative for the
    # interp; on HW the register tells the Q7 how many to transfer. Using
    # the padded max means trailing -1s are "transferred"?? -- instead pad
    # each core's run to the call size with idx 0 beyond its own count is
    # wrong; so: make every core's valid count EQUAL by padding shorter
    # cores with repeats of index 0 up to the max count, then -1 to the
    # call boundary.
    cnt_cs = counts  # [N_CORES, TILES*2]
    for t2 in range(TILES * 2):
        mx = int(cnt_cs[:, t2].max())
        if mx == 0 and sizes[t2] > 0:
            mx = 1
        s0v = int(starts[t2])
        for c2 in range(N_CORES):
            k2 = int(cnt_cs[c2, t2])
            if k2 < mx:
                IDX[c2, s0v + k2:s0v + mx] = 0
    valid = np.zeros(TILES * 2, dtype=np.int64)
    for t2 in range(TILES * 2):
        mx = int(cnt_cs[:, t2].max())
        if mx == 0 and sizes[t2] > 0:
            mx = 1
        valid[t2] = mx

    nL = (sizes.reshape(TILES, 2)[:, 0] // 128).tolist()
    nH = (sizes.reshape(TILES, 2)[:, 1] // 128).tolist()
    MAXBLK = int(max(nL[t] + nH[t] for t in range(TILES)))

    # gather-call list + wrapped idx tensor. One call per (tile, half);
    # the queue is assigned at build time as parity*2 + half so that at
    # most ONE triggered call is ever outstanding per queue (the 16
    # per-engine sem increments of two in-flight calls on one queue would
    # otherwise be indistinguishable -> torn-read race on msgs).
    idx_cols = TOT // 16
    IDXW = np.zeros((N_CORES, 128, idx_cols), dtype=np.int16)
    colp = 0
    call_list = []  # (tile, half, n_slots, idx_col_off, blk_off, n_valid)
    for t in range(TILES):
        blk_off = 0
        for h in (0, 1):
            n = int(sizes[t * 2 + h])
            if n == 0:
                continue
            s0 = int(starts[t * 2 + h])
            seg_idx = IDX[:, s0:s0 + n]
            IDXW[:, 0:16, colp:colp + n // 16] = (
                seg_idx.reshape(N_CORES, n // 16, 16).transpose(0, 2, 1))
            nv = int(valid[t * 2 + h])
            call_list.append((t, h, n, colp, blk_off, nv))
            colp += n // 16
            blk_off += n // 128
    IDXW[:, 16:128, :] = np.tile(IDXW[:, 0:16, :], (1, 7, 1))
    assert colp == idx_cols

    WT_pm = np.ascontiguousarray(
        WT.reshape(N_CORES, TOT // 128, 128, 128).transpose(0, 2, 1, 3)
        .reshape(N_CORES, 128, (TOT // 128) * 128))

    node_ps = _psrow(np.arange(N, dtype=np.int64))
    cfg = dict(N=N, E=E, SLICE=SLICE, TILES=TILES, ROWPAD=ROWPAD, PN=PN,
               LOW=LOW, TOT=TOT, MAXBLK=MAXBLK, call_list=call_list,
               nL=nL, nH=nH, chunk_cnt=cnt, chunk_off=coff, chunk_base=cbase,
               node_ps=node_ps)
    return cfg, IDXW, WT_pm


def _preprocess_values(cfg, x, s_local):
    """x / s_local dependent arrays (recomputed every call)."""
    B, N, F = x.shape
    COLS = B * F
    SLICE, TILES, ROWPAD, PN = cfg["SLICE"], cfg["TILES"], cfg["ROWPAD"], cfg["PN"]

    xb = np.ascontiguousarray(np.asarray(x, np.float32).transpose(1, 0, 2)
                              .reshape(N, COLS)).astype(ml_dtypes.bfloat16)
    h0 = np.zeros((PN, COLS), dtype=ml_dtypes.bfloat16)
    h0[cfg["node_ps"]] = xb
    xs = np.zeros((N_CORES * ROWPAD, COLS), dtype=ml_dtypes.bfloat16)
    for c in range(N_CORES):
        xs[c * ROWPAD:c * ROWPAD + SLICE] = xb[c * SLICE:(c + 1) * SLICE]
    x_slice = np.ascontiguousarray(xs.reshape(N_CORES, ROWPAD, COLS))
    x_slice_pm = np.ascontiguousarray(
        x_slice.reshape(N_CORES, TILES, 128, COLS).transpose(0, 2, 1, 3)
        .reshape(N_CORES, 128, TILES * COLS))

    s_pm = np.zeros((N_CORES, 128, TILES * B), dtype=np.float32)
    s_t = np.asarray(s_local, dtype=np.float32)
    for c in range(N_CORES):
        sl = np.zeros((ROWPAD, B), dtype=np.float32)
        sl[:SLICE] = s_t[:, c * SLICE:(c + 1) * SLICE].T
        s_pm[c] = sl.reshape(TILES, 128, B).transpose(1, 0, 2).reshape(128, TILES * B)
    return dict(h0=h0, x_slice=x_slice, x_slice_pm=x_slice_pm, s_pm=s_pm,
                B=B, F=F, COLS=COLS)


# ---------------------------------------------------------------- bass build


def _build(cfg, B, F, K1):
    COLS = B * F
    TILES, ROWPAD, PN = cfg["TILES"], cfg["ROWPAD"], cfg["PN"]
    LOW, TOT, MAXBLK = cfg["LOW"], cfg["TOT"], cfg["MAXBLK"]
    call_list = cfg["call_list"]
    nL, nH = cfg["nL"], cfg["nH"]
    c_cnt = [int(v) for v in cfg["chunk_cnt"]]
    c_off = [int(v) for v in cfg["chunk_off"]]
    c_base = [int(v) for v in cfg["chunk_base"]]
    chunk_of_tile = [j for j in range(NCHUNK) for _ in range(c_cnt[j])]
    # AG chunk j fires after the gathers of this tile are issued
    fire_after_tile = {t: [] for t in range(TILES)}
    for j in range(NCHUNK):
        ft = min(c_off[j] + c_cnt[j] - 1 + AG_SLACK, TILES - 1)
        fire_after_tile[ft].append(j)

    nc = bacc.Bacc("TRN2", debug=False, num_swdge_queues=NQ,
                   dynamic_dma_scratch_size=32768)

    h0_ext = nc.declare_dram_parameter("h0", [PN, COLS], BF16, isOutput=False)
    idx_ext = nc.declare_dram_parameter("idxw", [128, TOT // 16], I16, isOutput=False)
    wt_ext = nc.declare_dram_parameter("wt", [128, (TOT // 128) * 128], BF16, isOutput=False)
    xs_ext = nc.declare_dram_parameter("x_slice", [ROWPAD, COLS], BF16, isOutput=False)
    xspm_ext = nc.declare_dram_parameter("x_slice_pm", [128, TILES * COLS], BF16, isOutput=False)
    s_ext = nc.declare_dram_parameter("s_pm", [128, TILES * B], F32, isOutput=False)
    th_ext = nc.declare_dram_parameter("theta", [K1 * F, 2 * F], BF16, isOutput=False)
    bias_ext = nc.declare_dram_parameter("bias2", [1, 2 * F], BF16, isOutput=False)
    # partition-major output: row = bi*128 + d, col = t*F + f, so one store
    # covers 4 tiles contiguously per partition (fewer DMA descriptors)
    out_ext = nc.declare_dram_parameter("out", [B * 128, TILES * F], F32, isOutput=True)
    GRPB = (TILES + 3) // 4  # out-store groups per batch
    if DEBUG_DUMP:
        dbg1_ext = nc.declare_dram_parameter("dbg1", [PN, COLS], BF16, isOutput=True)
        dbg3_ext = nc.declare_dram_parameter("dbg3", [ROWPAD, COLS], BF16, isOutput=True)
        dbg0_ext = nc.declare_dram_parameter("dbg0", [ROWPAD, COLS], BF16, isOutput=True)

    t_sl = [None,
            nc.dram_tensor("t1s", [ROWPAD, COLS], BF16),
            nc.dram_tensor("t2s", [ROWPAD, COLS], BF16),
            nc.dram_tensor("t3s", [ROWPAD, COLS], BF16)]
    h_full = [None,
              nc.dram_tensor("h1f", [PN, COLS], BF16, addr_space="Shared"),
              nc.dram_tensor("h2f", [PN, COLS], BF16, addr_space="Shared")]
    warm_in = nc.dram_tensor("warm_in", [128, 64], BF16)
    warm_out = nc.dram_tensor("warm_out", [N_CORES * 128, 64], BF16,
                              addr_space="Shared")
    groups = [list(range(N_CORES))]

    calls_per_tile = {t: [] for t in range(TILES)}
    for (t, h, n, coff, boff, nv) in call_list:
        calls_per_tile[t].append((h, n, coff, boff, nv))

    tile_blk0 = []
    acc = 0
    for t in range(TILES):
        tile_blk0.append(acc)
        acc += nL[t] + nH[t]
    tile_nblk = [nL[t] + nH[t] for t in range(TILES)]

    # cumulative per-queue gather-sem / W-sem targets per (step, tile).
    # Queue = parity*2 + half; every call on queue q bumps gq[q] by 16 at
    # DMA completion, and at most one triggered call is in flight per queue.
    gtgt = {}
    wtgt = {}
    _g = [0, 0, 0, 0]
    _w = [0, 0]
    for step in range(3):
        for t in range(TILES):
            b = (step * TILES + t) % 2
            for (h, n, coff, boff, nv) in calls_per_tile[t]:
                _g[b * 2 + h] += 16
            gtgt[(step, t)] = tuple(_g)
            _w[b] += 16
            wtgt[(step, t)] = _w[b]

    from contextlib import ExitStack
    _es = ExitStack()
    with _es:
        sem = lambda n: _es.enter_context(nc.semaphore(n))
        sbuf = lambda n, s, d: _es.enter_context(nc.sbuf_tensor(n, s, d))
        idxS = sem("idxS"); xpmS = sem("xpmS")
        gq = [sem(f"gq{i}") for i in range(4)]
        pq = [sem(f"pq{i}") for i in range(4)]
        wSA = sem("wSA"); wSB = sem("wSB"); mmS = sem("mmS"); mmL = sem("mmL")
        evS = sem("evS"); ccS = sem("ccS"); onesS = sem("onesS")
        stC = [sem(f"stC{i}") for i in range(NCHUNK)]
        msS = sem("msS"); p2L = sem("p2L"); p2mm = sem("p2mm"); p2ev = sem("p2ev"); p2cp = sem("p2cp")
        p2T = [sem("p2T0"), sem("p2T1")]
        p2st = [sem(f"p2st{i}") for i in range(4)]
        msgsA = sbuf("msgsA", [128, MAXBLK, COLS], BF16)
        msgsB = sbuf("msgsB", [128, MAXBLK, COLS], BF16)
        wbufA = sbuf("wbufA", [128, MAXBLK, 128], BF16)
        wbufB = sbuf("wbufB", [128, MAXBLK, 128], BF16)
        idxs = sbuf("idxs", [128, TOT // 16], I16)
        arena0 = sbuf("arena0", [128, TILES * COLS], BF16)
        arena1 = sbuf("arena1", [128, TILES * COLS], BF16)
        ssb = sbuf("ssb", [128, TILES * B], F32)
        thsb = sbuf("thsb", [128, K1, 2 * F], BF16)
        ones1 = sbuf("ones1", [1, 128], BF16)
        bias_sb = sbuf("bias_sb", [1, 2 * F], BF16)
        outsb = [sbuf(f"outsb{i}", [128, 4 * F], F32) for i in range(2)]
        psA = _es.enter_context(nc.psum_tensor("psA", [128, COLS], F32))
        psB = _es.enter_context(nc.psum_tensor("psB", [128, COLS], F32))
        ps2 = [_es.enter_context(nc.psum_tensor(f"ps2{i}", [128, 2 * F], F32))
               for i in range(4)]
        msgs = [msgsA, msgsB]
        wbuf = [wbufA, wbufB]
        psum = [psA, psB]
        arenas = [arena0, arena1]
        # arena roles: arena0 = x -> T2 (in place at step 1) -> Tt[0:2]
        #              arena1 = T1 -> T3 (in place at step 2? no: cur list) -> Tt[2:4]
        prev_arena = [None, arena0, arena1]
        cur_arena = [arena1, arena0, arena1]   # steps 1,2 update in place
        wsem = [wSA, wSB]
        step_src = [h0_ext, h_full[1], h_full[2]]

        # which step stored which arena most recently, for overwrite waits:
        # cur_arena[1] = arena0 written at step1 (stores at step1);
        # cur_arena[2] = arena0 again -> step2 evac tile t must wait step1's
        # store of tile t: stS >= 16*(1*TILES + t + 1).

        # ---------------- phase 1
        with nc.Block() as blk:

            @blk.gpsimd
            def _(gpsimd):
                gpsimd.dma_start(out=idxs[:, :], in_=idx_ext[:, :]).then_inc(idxS, 16)
                gpsimd.dma_start(out=arena0[:, :], in_=xspm_ext[:, :]).then_inc(xpmS, 16)
                # dummy collective: absorbs ncfw first-collective warm-up
                # while the initial loads run (nothing waits on it; later
                # collectives queue behind it in straight-line order)
                gpsimd.collective_compute(
                    "AllGather",
                    mybir.AluOpType.bypass,
                    replica_groups=groups,
                    ins=[warm_in.ap().opt()],
                    outs=[warm_out.ap().opt()],
                ).then_inc(ccS)
                gpsimd.wait_ge(msS, 2)
                gpsimd.wait_ge(idxS, 16)

                NG = 3 * TILES
                prep_cnt = [0, 0, 0, 0]          # preps issued per queue
                trig_cnt = [0, 0, 0, 0]          # triggers issued per queue
                prep_tgt = {}                    # (g, q) -> prep_cnt to wait for

                def prep_tile(gp):
                    stepp, tp = gp // TILES, gp % TILES
                    bp = gp % 2
                    src_t = step_src[stepp]
                    for (h, n, coff, boff, nv) in calls_per_tile[tp]:
                        q = bp * 2 + h
                        src_ap = src_t[0:LOW, :] if h == 0 else src_t[LOW:PN, :]
                        gpsimd.dma_gather(
                            msgs[bp][:, boff:boff + n // 128, :],
                            src_ap,
                            idxs[:, coff:coff + n // 16],
                            n, nv, COLS,
                            single_packet=False,
                            queue_num=q,
                            prepare_only=True,
                            sem=gq[q],
                        ).then_inc(pq[q], 1)
                        prep_cnt[q] += 1
                        prep_tgt[(gp, q)] = prep_cnt[q]

                PRE = 2
                for gp in range(min(PRE, NG)):
                    prep_tile(gp)
                g = 0
                for step in range(3):
                    if step > 0:
                        # L calls read src rows [0, LOW) only -- covered by
                        # AG chunks 0..2 (chunk 3 starts above LOW). Fire the
                        # first two tiles' L drains under the final chunk's
                        # AllGather; H calls wait for the full gather source.
                        # (ccS: +1 for the warm-up collective; chunks done
                        # in issue order since collectives serialize.)
                        gpsimd.wait_ge(ccS, step * NCHUNK)
                        for tt in (0, 1):
                            gg = step * TILES + tt
                            bb = gg % 2
                            # msgs[bb] reuse: matmul gg-2 (prev step) done
                            gpsimd.wait_ge(mmS, gg - 1)
                            for (h, n, coff, boff, nv) in calls_per_tile[tt]:
                                if h == 0:
                                    q = bb * 2
                                    gpsimd.wait_ge(pq[q], prep_tgt[(gg, q)])
                                    gpsimd.trigger_dma(count=1, queue_num=q)
                                    trig_cnt[q] += 1
                        gpsimd.wait_ge(ccS, step * NCHUNK + 1)
                        for tt in (0, 1):
                            gg = step * TILES + tt
                            bb = gg % 2
                            for (h, n, coff, boff, nv) in calls_per_tile[tt]:
                                if h == 1:
                                    q = bb * 2 + 1
                                    gpsimd.wait_ge(pq[q], prep_tgt[(gg, q)])
                                    gpsimd.trigger_dma(count=1, queue_num=q)
                                    trig_cnt[q] += 1
                    for t in range(TILES):
                        b = g % 2
                        pretrig = step > 0 and t < 2
                        for (h, n, coff, boff, nv) in calls_per_tile[t]:
                            if pretrig:
                                continue
                            q = b * 2 + h
                            if g >= 2:
                                # L region frees once matmul g-2's L blocks
                                # retire (mmL) -- but only if this tile's L
                                # region does not spill into g-2's H blocks
                                # (nL varies per tile). H and spilling L need
                                # the full group (mmS).
                                tp2 = (g - 2) % TILES
                                early = h == 0 and nL[t] <= nL[tp2]
                                gpsimd.wait_ge(mmL if early else mmS, g - 1)
                            gpsimd.wait_ge(pq[q], prep_tgt[(g, q)])
                            gpsimd.trigger_dma(count=1, queue_num=q)
                            trig_cnt[q] += 1
                        g += 1
                        if g + PRE - 1 < NG:
                            prep_tile(g + PRE - 1)
                        if step < 2:
                            for j in fire_after_tile[t]:
                                r0 = c_off[j] * 128
                                r1 = (c_off[j] + c_cnt[j]) * 128
                                gpsimd.wait_ge(
                                    stC[j], 16 * c_cnt[j] * (step + 1))
                                gpsimd.collective_compute(
                                    "AllGather",
                                    mybir.AluOpType.bypass,
                                    replica_groups=groups,
                                    ins=[t_sl[step + 1][r0:r1, :].opt()],
                                    outs=[h_full[step + 1][
                                        c_base[j]:c_base[j] + N_CORES * (r1 - r0),
                                        :].opt()],
                                ).then_inc(ccS)

            @blk.sync
            def _(sync):
                for step in range(3):
                    for t in range(TILES):
                        g = step * TILES + t
                        b = g % 2
                        if g >= 2:
                            sync.wait_ge(mmS, g - 1)
                        nb = tile_nblk[t]
                        blk0 = tile_blk0[t]
                        sync.dma_start(
                            out=wbuf[b][:, 0:nb, :],
                            in_=wt_ext[:, blk0 * 128:(blk0 + nb) * 128],
                        ).then_inc(wsem[b], 16)

            @blk.tensor
            def _(tensor):
                prev_tgt = [0, 0, 0, 0]
                for step in range(3):
                    for t in range(TILES):
                        g = step * TILES + t
                        b = g % 2
                        if g >= 2:
                            # psum bank reuse: vector must have evacuated g-2
                            tensor.wait_ge(evS, g - 1)
                        tgts = gtgt[(step, t)]
                        tensor.wait_ge(wsem[b], wtgt[(step, t)])
                        nb = tile_nblk[t]
                        nbL = nL[t]
                        ins = None
                        # L blocks as soon as the L gather lands; H blocks
                        # (usually still draining) waited on separately.
                        if tgts[b * 2] > prev_tgt[b * 2]:
                            tensor.wait_ge(gq[b * 2], tgts[b * 2])
                        for blkno in range(nbL):
                            ins = tensor.matmul(
                                psum[b][:, :],
                                wbuf[b][:, blkno, :],
                                msgs[b][:, blkno, :],
                                start=(blkno == 0),
                                stop=(blkno == nb - 1),
                            )
                        ins.then_inc(mmL, 1)
                        if nb > nbL:
                            if tgts[b * 2 + 1] > prev_tgt[b * 2 + 1]:
                                tensor.wait_ge(gq[b * 2 + 1], tgts[b * 2 + 1])
                            for blkno in range(nbL, nb):
                                ins = tensor.matmul(
                                    psum[b][:, :],
                                    wbuf[b][:, blkno, :],
                                    msgs[b][:, blkno, :],
                                    start=False,
                                    stop=(blkno == nb - 1),
                                )
                        prev_tgt = list(tgts)
                        ins.then_inc(mmS, 1)

            @blk.vector
            def _(vector):
                vector.memset(ones1[:, :], 1.0).then_inc(onesS, 1)
                vector.memset(msgsA[:, :, :], 0.0).then_inc(msS, 1)
                vector.memset(msgsB[:, :, :], 0.0).then_inc(msS, 1)
                for step in range(3):
                    prev = prev_arena[step]
                    cur = cur_arena[step]
                    if step == 1:
                        vector.wait_ge(xpmS, 16)
                    for t in range(TILES):
                        b = (step * TILES + t) % 2
                        vector.wait_ge(mmS, step * TILES + t + 1)
                        dst = cur[:, t * COLS:(t + 1) * COLS]
                        if step == 0:
                            vector.tensor_scalar_mul(dst, psum[b][:, :], 1.0) \
                                .then_inc(evS, 1)
                        else:
                            vector.scalar_tensor_tensor(
                                dst,
                                psum[b][:, :],
                                2.0,
                                prev[:, t * COLS:(t + 1) * COLS],
                                op0=mybir.AluOpType.mult,
                                op1=mybir.AluOpType.subtract,
                            ).then_inc(evS, 1)

            @blk.scalar
            def _(scalar):
                for step in range(3):
                    cur = cur_arena[step]
                    for t in range(TILES):
                        scalar.wait_ge(evS, step * TILES + t + 1)
                        scalar.dma_start(
                            out=t_sl[step + 1][t * 128:(t + 1) * 128, :],
                            in_=cur[:, t * COLS:(t + 1) * COLS],
                        ).then_inc(stC[chunk_of_tile[t]], 16)

        # ---------------- phase 2
        with nc.Block() as blk2:

            @blk2.sync
            def _(sync):
                # theta/s/bias loads do not depend on phase-1 state
                sync.dma_start(out=ssb[:, :], in_=s_ext[:, :]).then_inc(p2L, 16)
                sync.dma_start(
                    out=thsb[:, :, :],
                    in_=th_ext[:, :].rearrange("(k f) o -> f k o", k=K1),
                ).then_inc(p2L, 16)
                sync.dma_start(out=bias_sb[:, :], in_=bias_ext[:, :]).then_inc(p2L, 16)
                for j in range(NCHUNK):
                    sync.wait_ge(stC[j], 16 * c_cnt[j] * 3)
                srcs = [xs_ext, t_sl[1], t_sl[2], t_sl[3]]
                # batch-parity double-buffered staging: arenas[bi%2] holds
                # the 4 transposed k-slices of batch bi (exactly 4*ROWPAD
                # cols), so batch bi+1 transposes overlap batch bi matmuls.
                for bi in range(B):
                    if bi >= 2:
                        sync.wait_ge(p2mm, (bi - 1) * TILES)
                    for k in range(K1):
                        dst = arenas[bi % 2][:, k * ROWPAD:k * ROWPAD + ROWPAD]
                        sync.dma_start_transpose(
                            dst, srcs[k][:, bi * F:(bi + 1) * F],
                        ).then_inc(p2T[bi % 2], 16)

            @blk2.tensor
            def _(tensor):
                tensor.wait_ge(onesS, 1)
                tensor.wait_ge(p2L, 48)
                for bi in range(B):
                    tensor.wait_ge(p2T[bi % 2], 16 * K1 * (bi // 2 + 1))
                    for t in range(TILES):
                        i = bi * TILES + t
                        pb = i % 4
                        if i >= 4:
                            tensor.wait_ge(p2ev, i - 3)
                        for k in range(K1):
                            src = arenas[bi % 2][:, k * ROWPAD + t * 128:
                                                 k * ROWPAD + (t + 1) * 128]
                            tensor.matmul(
                                ps2[pb][:, :],
                                src,
                                thsb[:, k, :],
                                start=(k == 0),
                                stop=False,
                            )
                        tensor.matmul(
                            ps2[pb][:, :],
                            ones1[:, :],
                            bias_sb[:, :],
                            start=False,
                            stop=True,
                        ).then_inc(p2mm, 1)

            @blk2.vector
            def _(vector):
                for bi in range(B):
                    for t in range(TILES):
                        i = bi * TILES + t
                        pb = i % 4
                        grp = bi * GRPB + t // 4
                        sl = t % 4
                        vector.wait_ge(p2mm, i + 1)
                        if sl == 0 and grp >= 2:
                            vector.wait_ge(p2st[grp % 2], 16 * (grp // 2))
                        dst = outsb[grp % 2][:, sl * F:(sl + 1) * F]
                        vector.tensor_scalar_mul(
                            dst, ps2[pb][:, 0:F], 1.0) \
                            .then_inc(p2cp, 1)
                        vector.wait_ge(p2cp, i + 1)
                        vector.scalar_tensor_tensor(
                            dst,
                            ps2[pb][:, F:2 * F],
                            ssb[:, (t * B + bi):(t * B + bi) + 1],
                            dst,
                            op0=mybir.AluOpType.mult,
                            op1=mybir.AluOpType.add,
                        ).then_inc(p2ev, 1)

            @blk2.scalar
            def _(scalar):
                for bi in range(B):
                    for t0 in range(0, TILES, 4):
                        gsz = min(4, TILES - t0)
                        grp = bi * GRPB + t0 // 4
                        i_last = bi * TILES + t0 + gsz - 1
                        scalar.wait_ge(p2ev, i_last + 1)
                        scalar.dma_start(
                            out=out_ext[bi * 128:(bi + 1) * 128,
                                        t0 * F:(t0 + gsz) * F],
                            in_=outsb[grp % 2][:, 0:gsz * F],
                        ).then_inc(p2st[grp % 2], 16)

        if DEBUG_DUMP:
            with nc.Block() as blk3:
                @blk3.sync
                def _(sync):
                    for _q in range(4):
                        sync.wait_ge(p2st[_q], 16)
                    base = 16
                    for i in range(PN // 128):
                        buf = msgs[i % 2][:, 0, :]
                        sync.dma_start(out=buf, in_=h_full[1][i * 128:(i + 1) * 128, :]).then_inc(idxS, 16)
                        sync.wait_ge(idxS, base + 16)
                        sync.dma_start(out=dbg1_ext[i * 128:(i + 1) * 128, :], in_=buf).then_inc(idxS, 16)
                        sync.wait_ge(idxS, base + 32)
                        base += 32
                    for i in range(ROWPAD // 128):
                        buf = msgs[i % 2][:, 0, :]
                        sync.dma_start(out=buf, in_=t_sl[3][i * 128:(i + 1) * 128, :]).then_inc(idxS, 16)
                        sync.wait_ge(idxS, base + 16)
                        sync.dma_start(out=dbg3_ext[i * 128:(i + 1) * 128, :], in_=buf).then_inc(idxS, 16)
                        sync.wait_ge(idxS, base + 32)
                        base += 32
                    for i in range(ROWPAD // 128):
                        buf = msgs[i % 2][:, 0, :]
                        sync.dma_start(out=buf, in_=t_sl[1][i * 128:(i + 1) * 128, :]).then_inc(idxS, 16)
                        sync.wait_ge(idxS, base + 16)
                        sync.dma_start(out=dbg0_ext[i * 128:(i + 1) * 128, :], in_=buf).then_inc(idxS, 16)
                        sync.wait_ge(idxS, base + 32)
                        base += 32

    nc.finalize()
    return nc


# ---------------------------------------------------------------- entry

_cache = {}


def _get_graph(N, B, F, K1, edge_index, edge_attr):
    key = (N, B, F, K1,
           hash(np.asarray(edge_index).tobytes()),
           hash(np.asarray(edge_attr).tobytes()))
    if key in _cache:
        return _cache[key]
    cfg, IDXW, WT_pm = _preprocess_edges(N, edge_index, edge_attr)
    nc = _build(cfg, B, F, K1)
    _cache.clear()
    _cache[key] = (cfg, IDXW, WT_pm, nc)
    return _cache[key]


def kernel(x, edge_index, edge_attr, s_local, Theta0, Theta1, bias):
    x = np.asarray(x)
    B, N, F = x.shape
    K1 = np.asarray(Theta0).shape[0]
    cfg, IDXW, WT_pm, nc = _get_graph(N, B, F, K1, edge_index, edge_attr)
    vals = _preprocess_values(cfg, x, s_local)
    SLICE, ROWPAD = cfg["SLICE"], cfg["ROWPAD"]
    COLS = vals["COLS"]

    th = np.concatenate([np.asarray(Theta0, np.float32),
                         np.asarray(Theta1, np.float32)], axis=2)
    th_b = np.ascontiguousarray(th).astype(ml_dtypes.bfloat16).reshape(K1 * F, 2 * F)
    bias2 = np.zeros((1, 2 * F), dtype=ml_dtypes.bfloat16)
    bias2[0, :F] = np.asarray(bias, np.float32).astype(ml_dtypes.bfloat16)

    in_maps = []
    for c in range(N_CORES):
        in_maps.append({
            "h0": vals["h0"],
            "idxw": np.ascontiguousarray(IDXW[c]),
            "wt": np.ascontiguousarray(WT_pm[c]),
            "x_slice": np.ascontiguousarray(vals["x_slice"][c]),
            "x_slice_pm": np.ascontiguousarray(vals["x_slice_pm"][c]),
            "s_pm": np.ascontiguousarray(vals["s_pm"][c]),
            "theta": th_b,
            "bias2": bias2,
        })

    trace = _maybe_install_ntff_hook()
    import tempfile
    tdir = tempfile.mkdtemp() if trace else None
    res = run_bass_kernel_spmd(nc, in_maps, core_ids=list(range(N_CORES)),
                               trace=trace, tmpdir=tdir)
    global last_exec_time_ns, last_trace_dir
    last_exec_time_ns = res.exec_time_ns
    last_trace_dir = tdir
    TILES = cfg["TILES"]
    out = np.empty((B, N, F), dtype=np.float32)
    for c in range(N_CORES):
        oc = res.results[c]["out"]  # [B*128, TILES*F] partition-major
        ocr = oc.reshape(B, 128, TILES, F).transpose(0, 2, 1, 3) \
                .reshape(B, ROWPAD, F)
        out[:, c * SLICE:(c + 1) * SLICE, :] = ocr[:, :SLICE]
    return out



# revision 53
# speedup vs baseline: 1.0272x; 1.0212x over previous
"""AdaptiveGraphWaveletConv Trainium2 kernel (8 NeuronCores, SPMD).

Math (reference):
    mp(h)[d] = sum_{e: dst_e=d} w_e * h[src_e]          (per batch)
    T_0 = x; T_1 = mp(x); T_k = 2*mp(T_{k-1}) - T_{k-2} (K=3)
    out = sum_k T_k @ Theta0_k + s_local * (sum_k T_k @ Theta1_k) + bias

Strategy:
  - 8-way destination-node split (6250 nodes/core), all 4 batches fused into
    512 bf16 feature columns -> gather rows are 1KB (latency-optimal for the
    SWDGE dma_gather path, 4 queues).
  - Message passing per Chebyshev step: dma_gather h[src] rows from local HBM,
    TensorE scatter-reduce with host-precomputed weighted one-hot W^T blocks
    (lhsT=[128 edge slots, 128 local dst], rhs=gathered msgs [128, 512]),
    PSUM accumulation per 128-dst tile, VectorE Chebyshev update.
  - 8-rank AllGather replicates T_k to every core's HBM between steps.
  - Phase 2: DMA-transpose T_k slices, matmul against [Theta0|Theta1],
    bias via ones-row matmul, out0 + s*out1 on VectorE.

The per-(tile, src-half) slot counts are normalized to the max over all 8
cores so every core runs the identical instruction stream (SPMD), padding
with (idx=0, w=0) slots.
"""

import sys

sys.path.insert(0, "/opt/trn_rl_repo")

import os

import numpy as np
import ml_dtypes

from concourse import bass, bacc, mybir
from concourse.bass_utils import run_bass_kernel_spmd

last_exec_time_ns = None
last_trace_dir = None


def _maybe_install_ntff_hook():
    if not os.environ.get("BASS_KERNEL_TRACE"):
        return False
    import types
    import antenv
    if not hasattr(antenv, "axon_hooks"):
        _m = types.ModuleType("antenv.axon_hooks")
        _m._hook = None
        def set_axon_ntff_profile_hook(h): _m._hook = h
        def get_axon_ntff_profile_hook(): return _m._hook
        _m.set_axon_ntff_profile_hook = set_axon_ntff_profile_hook
        _m.get_axon_ntff_profile_hook = get_axon_ntff_profile_hook
        sys.modules["antenv.axon_hooks"] = _m
        antenv.axon_hooks = _m
        try:
            from trn_agent_boot.trn_boot import _ntff_profile_via_ctypes
            set_axon_ntff_profile_hook(
                _ntff_profile_via_ctypes("/opt/axon/libaxon_pjrt.so"))
        except Exception:
            return False
    return True

BF16 = mybir.dt.bfloat16
F32 = mybir.dt.float32
I16 = mybir.dt.int16

N_CORES = 8
NQ = 4  # SWDGE queues
LOW_CAP = 32768  # int16 index split (overridable for tests)
NCHUNK = 4  # AllGather chunks per step (overlap with step tail)
AG_SLACK = 5  # tiles of slack before waiting on a chunk's stores
DEBUG_DUMP = False  # add h1f/t3s debug outputs


# ---------------------------------------------------------------- host side


def _preprocess_edges(N, edge_index, edge_attr):
    """Edge-structure-dependent arrays (cacheable with the compiled graph)."""
    E = edge_index.shape[1]
    SLICE = N // N_CORES
    TILES = (SLICE + 127) // 128
    ROWPAD = TILES * 128
    PN = N_CORES * ROWPAD
    LOW = min(LOW_CAP, PN)

    dst = np.asarray(edge_index[0], dtype=np.int64)
    src = np.asarray(edge_index[1], dtype=np.int64)
    w = np.asarray(edge_attr, dtype=np.float32)

    # chunk-major global row layout for h buffers:
    #   [chunk j][core c][tile t-o_j][128 rows]
    # so each AllGather chunk writes one contiguous range of h_full.
    base_nt = TILES // NCHUNK
    cnt = np.full(NCHUNK, base_nt, dtype=np.int64)
    cnt[:TILES - base_nt * NCHUNK] += 1
    coff = np.zeros(NCHUNK, dtype=np.int64)
    np.cumsum(cnt[:-1], out=coff[1:])
    chunk_of_tile = np.repeat(np.arange(NCHUNK), cnt)
    cbase = np.zeros(NCHUNK, dtype=np.int64)
    np.cumsum((N_CORES * cnt * 128)[:-1], out=cbase[1:])

    def _psrow(node):
        c_s = node // SLICE
        r_s = node % SLICE
        t_s = r_s // 128
        d_s = r_s % 128
        j_s = chunk_of_tile[t_s]
        return cbase[j_s] + c_s * cnt[j_s] * 128 + (t_s - coff[j_s]) * 128 + d_s

    core = dst // SLICE
    tile = (dst % SLICE) // 128
    d_loc = (dst % SLICE) % 128
    ps = _psrow(src)
    half = (ps >= LOW).astype(np.int64)

    seg = tile * 2 + half
    seg_key = core * (TILES * 2) + seg
    counts = np.bincount(seg_key, minlength=N_CORES * TILES * 2) \
        .reshape(N_CORES, TILES * 2)
    sizes = counts.max(axis=0)
    sizes = ((sizes + 127) // 128) * 128
    # every tile needs >= 1 block so its PSUM group exists
    st = sizes.reshape(TILES, 2)
    st[st.sum(axis=1) == 0, 0] = 128
    sizes = st.reshape(-1)
    starts = np.zeros(TILES * 2 + 1, dtype=np.int64)
    np.cumsum(sizes, out=starts[1:])
    TOT = int(starts[-1])

    order = np.lexsort((ps, seg, core))
    core_s, seg_s = core[order], seg[order]
    dloc_s, ps_s, half_s, w_s = d_loc[order], ps[order], half[order], w[order]
    seg_key_s = core_s * (TILES * 2) + seg_s
    run_counts = np.bincount(seg_key_s, minlength=N_CORES * TILES * 2)
    run_starts = np.zeros(N_CORES * TILES * 2 + 1, dtype=np.int64)
    np.cumsum(run_counts, out=run_starts[1:])
    rank_in_run = np.arange(E) - run_starts[seg_key_s]
    slot = starts[seg_s] + rank_in_run

    IDX = np.full((N_CORES, TOT), -1, dtype=np.int16)
    WT = np.zeros((N_CORES, TOT, 128), dtype=ml_dtypes.bfloat16)
    IDX[core_s, slot] = (ps_s - half_s * LOW).astype(np.int16)
    WT[core_s, slot, dloc_s] = w_s.astype(ml_dtypes.bfloat16)
    # per-(core, seg) valid counts; per-call num_idxs_reg = max over cores
    # must equal THIS core's count -> but SPMD needs one immediate; the HW
    # contract only requires num_idxs_reg == count of non-negative for the
    # interp; on HW the register tells the Q7 how many to transfer. Using
    # the padded max means trailing -1s are "transferred"?? -- instead pad
    # each core's run to the call size with idx 0 beyond its own count is
    # wrong; so: make every core's valid count EQUAL by padding shorter
    # cores with repeats of index 0 up to the max count, then -1 to the
    # call boundary.
    cnt_cs = counts  # [N_CORES, TILES*2]
    for t2 in range(TILES * 2):
        mx = int(cnt_cs[:, t2].max())
        if mx == 0 and sizes[t2] > 0:
            mx = 1
        s0v = int(starts[t2])
        for c2 in range(N_CORES):
            k2 = int(cnt_cs[c2, t2])
            if k2 < mx:
                IDX[c2, s0v + k2:s0v + mx] = 0
    valid = np.zeros(TILES * 2, dtype=np.int64)
    for t2 in range(TILES * 2):
        mx = int(cnt_cs[:, t2].max())
        if mx == 0 and sizes[t2] > 0:
            mx = 1
        valid[t2] = mx

    nL = (sizes.reshape(TILES, 2)[:, 0] // 128).tolist()
    nH = (sizes.reshape(TILES, 2)[:, 1] // 128).tolist()
    MAXBLK = int(max(nL[t] + nH[t] for t in range(TILES)))

    # gather-call list + wrapped idx tensor. One call per (tile, half);
    # the queue is assigned at build time as parity*2 + half so that at
    # most ONE triggered call is ever outstanding per queue (the 16
    # per-engine sem increments of two in-flight calls on one queue would
    # otherwise be indistinguishable -> torn-read race on msgs).
    idx_cols = TOT // 16
    IDXW = np.zeros((N_CORES, 128, idx_cols), dtype=np.int16)
    colp = 0
    call_list = []  # (tile, half, n_slots, idx_col_off, blk_off, n_valid)
    for t in range(TILES):
        blk_off = 0
        for h in (0, 1):
            n = int(sizes[t * 2 + h])
            if n == 0:
                continue
            s0 = int(starts[t * 2 + h])
            seg_idx = IDX[:, s0:s0 + n]
            IDXW[:, 0:16, colp:colp + n // 16] = (
                seg_idx.reshape(N_CORES, n // 16, 16).transpose(0, 2, 1))
            nv = int(valid[t * 2 + h])
            call_list.append((t, h, n, colp, blk_off, nv))
            colp += n // 16
            blk_off += n // 128
    IDXW[:, 16:128, :] = np.tile(IDXW[:, 0:16, :], (1, 7, 1))
    assert colp == idx_cols

    WT_pm = np.ascontiguousarray(
        WT.reshape(N_CORES, TOT // 128, 128, 128).transpose(0, 2, 1, 3)
        .reshape(N_CORES, 128, (TOT // 128) * 128))

    node_ps = _psrow(np.arange(N, dtype=np.int64))
    cfg = dict(N=N, E=E, SLICE=SLICE, TILES=TILES, ROWPAD=ROWPAD, PN=PN,
               LOW=LOW, TOT=TOT, MAXBLK=MAXBLK, call_list=call_list,
               nL=nL, nH=nH, chunk_cnt=cnt, chunk_off=coff, chunk_base=cbase,
               node_ps=node_ps)
    return cfg, IDXW, WT_pm


def _preprocess_values(cfg, x, s_local):
    """x / s_local dependent arrays (recomputed every call)."""
    B, N, F = x.shape
    COLS = B * F
    SLICE, TILES, ROWPAD, PN = cfg["SLICE"], cfg["TILES"], cfg["ROWPAD"], cfg["PN"]

    xb = np.ascontiguousarray(np.asarray(x, np.float32).transpose(1, 0, 2)
                              .reshape(N, COLS)).astype(ml_dtypes.bfloat16)
    h0 = np.zeros((PN, COLS), dtype=ml_dtypes.bfloat16)
    h0[cfg["node_ps"]] = xb
    xs = np.zeros((N_CORES * ROWPAD, COLS), dtype=ml_dtypes.bfloat16)
    for c in range(N_CORES):
        xs[c * ROWPAD:c * ROWPAD + SLICE] = xb[c * SLICE:(c + 1) * SLICE]
    x_slice = np.ascontiguousarray(xs.reshape(N_CORES, ROWPAD, COLS))
    x_slice_pm = np.ascontiguousarray(
        x_slice.reshape(N_CORES, TILES, 128, COLS).transpose(0, 2, 1, 3)
        .reshape(N_CORES, 128, TILES * COLS))

    s_pm = np.zeros((N_CORES, 128, TILES * B), dtype=np.float32)
    s_t = np.asarray(s_local, dtype=np.float32)
    for c in range(N_CORES):
        sl = np.zeros((ROWPAD, B), dtype=np.float32)
        sl[:SLICE] = s_t[:, c * SLICE:(c + 1) * SLICE].T
        s_pm[c] = sl.reshape(TILES, 128, B).transpose(1, 0, 2).reshape(128, TILES * B)
    return dict(h0=h0, x_slice=x_slice, x_slice_pm=x_slice_pm, s_pm=s_pm,
                B=B, F=F, COLS=COLS)


# ---------------------------------------------------------------- bass build


def _build(cfg, B, F, K1):
    COLS = B * F
    TILES, ROWPAD, PN = cfg["TILES"], cfg["ROWPAD"], cfg["PN"]
    LOW, TOT, MAXBLK = cfg["LOW"], cfg["TOT"], cfg["MAXBLK"]
    call_list = cfg["call_list"]
    nL, nH = cfg["nL"], cfg["nH"]
    c_cnt = [int(v) for v in cfg["chunk_cnt"]]
    c_off = [int(v) for v in cfg["chunk_off"]]
    c_base = [int(v) for v in cfg["chunk_base"]]
    chunk_of_tile = [j for j in range(NCHUNK) for _ in range(c_cnt[j])]
    # AG chunk j fires after the gathers of this tile are issued
    fire_after_tile = {t: [] for t in range(TILES)}
    for j in range(NCHUNK):
        ft = min(c_off[j] + c_cnt[j] - 1 + AG_SLACK, TILES - 1)
        fire_after_tile[ft].append(j)

    nc = bacc.Bacc("TRN2", debug=False, num_swdge_queues=NQ,
                   dynamic_dma_scratch_size=32768)

    h0_ext = nc.declare_dram_parameter("h0", [PN, COLS], BF16, isOutput=False)
    idx_ext = nc.declare_dram_parameter("idxw", [128, TOT // 16], I16, isOutput=False)
    wt_ext = nc.declare_dram_parameter("wt", [128, (TOT // 128) * 128], BF16, isOutput=False)
    xs_ext = nc.declare_dram_parameter("x_slice", [ROWPAD, COLS], BF16, isOutput=False)
    xspm_ext = nc.declare_dram_parameter("x_slice_pm", [128, TILES * COLS], BF16, isOutput=False)
    s_ext = nc.declare_dram_parameter("s_pm", [128, TILES * B], F32, isOutput=False)
    th_ext = nc.declare_dram_parameter("theta", [K1 * F, 2 * F], BF16, isOutput=False)
    bias_ext = nc.declare_dram_parameter("bias2", [1, 2 * F], BF16, isOutput=False)
    # partition-major output: row = bi*128 + d, col = t*F + f, so one store
    # covers 4 tiles contiguously per partition (fewer DMA descriptors)
    out_ext = nc.declare_dram_parameter("out", [B * 128, TILES * F], F32, isOutput=True)
    GRPB = (TILES + 3) // 4  # out-store groups per batch
    if DEBUG_DUMP:
        dbg1_ext = nc.declare_dram_parameter("dbg1", [PN, COLS], BF16, isOutput=True)
        dbg3_ext = nc.declare_dram_parameter("dbg3", [ROWPAD, COLS], BF16, isOutput=True)
        dbg0_ext = nc.declare_dram_parameter("dbg0", [ROWPAD, COLS], BF16, isOutput=True)

    t_sl = [None,
            nc.dram_tensor("t1s", [ROWPAD, COLS], BF16),
            nc.dram_tensor("t2s", [ROWPAD, COLS], BF16),
            nc.dram_tensor("t3s", [ROWPAD, COLS], BF16)]
    h_full = [None,
              nc.dram_tensor("h1f", [PN, COLS], BF16, addr_space="Shared"),
              nc.dram_tensor("h2f", [PN, COLS], BF16, addr_space="Shared")]
    warm_in = nc.dram_tensor("warm_in", [128, 64], BF16)
    warm_out = nc.dram_tensor("warm_out", [N_CORES * 128, 64], BF16,
                              addr_space="Shared")
    groups = [list(range(N_CORES))]

    calls_per_tile = {t: [] for t in range(TILES)}
    for (t, h, n, coff, boff, nv) in call_list:
        calls_per_tile[t].append((h, n, coff, boff, nv))

    tile_blk0 = []
    acc = 0
    for t in range(TILES):
        tile_blk0.append(acc)
        acc += nL[t] + nH[t]
    tile_nblk = [nL[t] + nH[t] for t in range(TILES)]

    # cumulative per-queue gather-sem / W-sem targets per (step, tile).
    # Queue = parity*2 + half; every call on queue q bumps gq[q] by 16 at
    # DMA completion, and at most one triggered call is in flight per queue.
    gtgt = {}
    wtgt = {}
    _g = [0, 0, 0, 0]
    _w = [0, 0]
    for step in range(3):
        for t in range(TILES):
            b = (step * TILES + t) % 2
            for (h, n, coff, boff, nv) in calls_per_tile[t]:
                _g[b * 2 + h] += 16
            gtgt[(step, t)] = tuple(_g)
            _w[b] += 16
            wtgt[(step, t)] = _w[b]

    from contextlib import ExitStack
    _es = ExitStack()
    with _es:
        sem = lambda n: _es.enter_context(nc.semaphore(n))
        sbuf = lambda n, s, d: _es.enter_context(nc.sbuf_tensor(n, s, d))
        idxS = sem("idxS"); xpmS = sem("xpmS")
        gq = [sem(f"gq{i}") for i in range(4)]
        pq = [sem(f"pq{i}") for i in range(4)]
        wSA = sem("wSA"); wSB = sem("wSB"); mmS = sem("mmS"); mmL = sem("mmL")
        evS = sem("evS"); ccS = sem("ccS"); onesS = sem("onesS")
        stC = [sem(f"stC{i}") for i in range(NCHUNK)]
        msS = sem("msS"); p2L = sem("p2L"); p2mm = sem("p2mm"); p2ev = sem("p2ev"); p2cp = sem("p2cp")
        p2T = [sem("p2T0"), sem("p2T1")]
        p2st = [sem(f"p2st{i}") for i in range(4)]
        msgsA = sbuf("msgsA", [128, MAXBLK, COLS], BF16)
        msgsB = sbuf("msgsB", [128, MAXBLK, COLS], BF16)
        wbufA = sbuf("wbufA", [128, MAXBLK, 128], BF16)
        wbufB = sbuf("wbufB", [128, MAXBLK, 128], BF16)
        idxs = sbuf("idxs", [128, TOT // 16], I16)
        arena0 = sbuf("arena0", [128, TILES * COLS], BF16)
        arena1 = sbuf("arena1", [128, TILES * COLS], BF16)
        ssb = sbuf("ssb", [128, TILES * B], F32)
        thsb = sbuf("thsb", [128, K1, 2 * F], BF16)
        ones1 = sbuf("ones1", [1, 128], BF16)
        bias_sb = sbuf("bias_sb", [1, 2 * F], BF16)
        outsb = [sbuf(f"outsb{i}", [128, 4 * F], F32) for i in range(2)]
        psA = _es.enter_context(nc.psum_tensor("psA", [128, COLS], F32))
        psB = _es.enter_context(nc.psum_tensor("psB", [128, COLS], F32))
        ps2 = [_es.enter_context(nc.psum_tensor(f"ps2{i}", [128, 2 * F], F32))
               for i in range(4)]
        msgs = [msgsA, msgsB]
        wbuf = [wbufA, wbufB]
        psum = [psA, psB]
        arenas = [arena0, arena1]
        # arena roles: arena0 = x -> T2 (in place at step 1) -> Tt[0:2]
        #              arena1 = T1 -> T3 (in place at step 2? no: cur list) -> Tt[2:4]
        prev_arena = [None, arena0, arena1]
        cur_arena = [arena1, arena0, arena1]   # steps 1,2 update in place
        wsem = [wSA, wSB]
        step_src = [h0_ext, h_full[1], h_full[2]]

        # which step stored which arena most recently, for overwrite waits:
        # cur_arena[1] = arena0 written at step1 (stores at step1);
        # cur_arena[2] = arena0 again -> step2 evac tile t must wait step1's
        # store of tile t: stS >= 16*(1*TILES + t + 1).

        # ---------------- phase 1
        with nc.Block() as blk:

            @blk.gpsimd
            def _(gpsimd):
                gpsimd.dma_start(out=idxs[:, :], in_=idx_ext[:, :]).then_inc(idxS, 16)
                gpsimd.dma_start(out=arena0[:, :], in_=xspm_ext[:, :]).then_inc(xpmS, 16)
                # dummy collective: absorbs ncfw first-collective warm-up
                # while the initial loads run (nothing waits on it; later
                # collectives queue behind it in straight-line order)
                gpsimd.collective_compute(
                    "AllGather",
                    mybir.AluOpType.bypass,
                    replica_groups=groups,
                    ins=[warm_in.ap().opt()],
                    outs=[warm_out.ap().opt()],
                ).then_inc(ccS)
                gpsimd.wait_ge(msS, 2)
                gpsimd.wait_ge(idxS, 16)

                NG = 3 * TILES
                prep_cnt = [0, 0, 0, 0]          # preps issued per queue
                trig_cnt = [0, 0, 0, 0]          # triggers issued per queue
                prep_tgt = {}                    # (g, q) -> prep_cnt to wait for

                def prep_tile(gp):
                    stepp, tp = gp // TILES, gp % TILES
                    bp = gp % 2
                    src_t = step_src[stepp]
                    for (h, n, coff, boff, nv) in calls_per_tile[tp]:
                        q = bp * 2 + h
                        src_ap = src_t[0:LOW, :] if h == 0 else src_t[LOW:PN, :]
                        gpsimd.dma_gather(
                            msgs[bp][:, boff:boff + n // 128, :],
                            src_ap,
                            idxs[:, coff:coff + n // 16],
                            n, nv, COLS,
                            single_packet=False,
                            queue_num=q,
                            prepare_only=True,
                            sem=gq[q],
                        ).then_inc(pq[q], 1)
                        prep_cnt[q] += 1
                        prep_tgt[(gp, q)] = prep_cnt[q]

                PRE = 2
                for gp in range(min(PRE, NG)):
                    prep_tile(gp)
                g = 0
                for step in range(3):
                    if step > 0:
                        # L calls read src rows [0, LOW) only -- covered by
                        # AG chunks 0..2 (chunk 3 starts above LOW). Fire the
                        # first two tiles' L drains under the final chunk's
                        # AllGather; H calls wait for the full gather source.
                        # (ccS: +1 for the warm-up collective; chunks done
                        # in issue order since collectives serialize.)
                        gpsimd.wait_ge(ccS, step * NCHUNK)
                        for tt in (0, 1):
                            gg = step * TILES + tt
                            bb = gg % 2
                            # msgs[bb] reuse: matmul gg-2 (prev step) done
                            gpsimd.wait_ge(mmS, gg - 1)
                            for (h, n, coff, boff, nv) in calls_per_tile[tt]:
                                if h == 0:
                                    q = bb * 2
                                    gpsimd.wait_ge(pq[q], prep_tgt[(gg, q)])
                                    gpsimd.trigger_dma(count=1, queue_num=q)
                                    trig_cnt[q] += 1
                        gpsimd.wait_ge(ccS, step * NCHUNK + 1)
                        for tt in (0, 1):
                            gg = step * TILES + tt
                            bb = gg % 2
                            for (h, n, coff, boff, nv) in calls_per_tile[tt]:
                                if h == 1:
                                    q = bb * 2 + 1
                                    gpsimd.wait_ge(pq[q], prep_tgt[(gg, q)])
                                    gpsimd.trigger_dma(count=1, queue_num=q)
                                    trig_cnt[q] += 1
                    for t in range(TILES):
                        b = g % 2
                        pretrig = step > 0 and t < 2
                        for (h, n, coff, boff, nv) in calls_per_tile[t]:
                            if pretrig:
                                continue
                            q = b * 2 + h
                            if g >= 2:
                                # L region frees once matmul g-2's L blocks
                                # retire (mmL) -- but only if this tile's L
                                # region does not spill into g-2's H blocks
                                # (nL varies per tile). H and spilling L need
                                # the full group (mmS).
                                tp2 = (g - 2) % TILES
                                early = h == 0 and nL[t] <= nL[tp2]
                                gpsimd.wait_ge(mmL if early else mmS, g - 1)
                            gpsimd.wait_ge(pq[q], prep_tgt[(g, q)])
                            gpsimd.trigger_dma(count=1, queue_num=q)
                            trig_cnt[q] += 1
                        g += 1
                        if g + PRE - 1 < NG:
                            prep_tile(g + PRE - 1)
                        if step < 2:
                            for j in fire_after_tile[t]:
                                r0 = c_off[j] * 128
                                r1 = (c_off[j] + c_cnt[j]) * 128
                                gpsimd.wait_ge(
                                    stC[j], 16 * c_cnt[j] * (step + 1))
                                gpsimd.collective_compute(
                                    "AllGather",
                                    mybir.AluOpType.bypass,
                                    replica_groups=groups,
                                    ins=[t_sl[step + 1][r0:r1, :].opt()],
                                    outs=[h_full[step + 1][
                                        c_base[j]:c_base[j] + N_CORES * (r1 - r0),
                                        :].opt()],
                                ).then_inc(ccS)

            @blk.sync
            def _(sync):
                for step in range(3):
                    for t in range(TILES):
                        g = step * TILES + t
                        b = g % 2
                        if g >= 2:
                            sync.wait_ge(mmS, g - 1)
                        nb = tile_nblk[t]
                        blk0 = tile_blk0[t]
                        sync.dma_start(
                            out=wbuf[b][:, 0:nb, :],
                            in_=wt_ext[:, blk0 * 128:(blk0 + nb) * 128],
                        ).then_inc(wsem[b], 16)

            @blk.tensor
            def _(tensor):
                prev_tgt = [0, 0, 0, 0]
                for step in range(3):
                    for t in range(TILES):
                        g = step * TILES + t
                        b = g % 2
                        if g >= 2:
                            # psum bank reuse: vector must have evacuated g-2
                            tensor.wait_ge(evS, g - 1)
                        tgts = gtgt[(step, t)]
                        tensor.wait_ge(wsem[b], wtgt[(step, t)])
                        nb = tile_nblk[t]
                        nbL = nL[t]
                        ins = None
                        # L blocks as soon as the L gather lands; H blocks
                        # (usually still draining) waited on separately.
                        if tgts[b * 2] > prev_tgt[b * 2]:
                            tensor.wait_ge(gq[b * 2], tgts[b * 2])
                        for blkno in range(nbL):
                            ins = tensor.matmul(
                                psum[b][:, :],
                                wbuf[b][:, blkno, :],
                                msgs[b][:, blkno, :],
                                start=(blkno == 0),
                                stop=(blkno == nb - 1),
                            )
                        ins.then_inc(mmL, 1)
                        if nb > nbL:
                            if tgts[b * 2 + 1] > prev_tgt[b * 2 + 1]:
                                tensor.wait_ge(gq[b * 2 + 1], tgts[b * 2 + 1])
                            for blkno in range(nbL, nb):
                                ins = tensor.matmul(
                                    psum[b][:, :],
                                    wbuf[b][:, blkno, :],
                                    msgs[b][:, blkno, :],
                                    start=False,
                                    stop=(blkno == nb - 1),
                                )
                        prev_tgt = list(tgts)
                        ins.then_inc(mmS, 1)

            @blk.vector
            def _(vector):
                vector.memset(ones1[:, :], 1.0).then_inc(onesS, 1)
                vector.memset(msgsA[:, :, :], 0.0).then_inc(msS, 1)
                vector.memset(msgsB[:, :, :], 0.0).then_inc(msS, 1)
                for step in range(3):
                    prev = prev_arena[step]
                    cur = cur_arena[step]
                    if step == 1:
                        vector.wait_ge(xpmS, 16)
                    for t in range(TILES):
                        b = (step * TILES + t) % 2
                        vector.wait_ge(mmS, step * TILES + t + 1)
                        dst = cur[:, t * COLS:(t + 1) * COLS]
                        if step == 0:
                            vector.tensor_scalar_mul(dst, psum[b][:, :], 1.0) \
                                .then_inc(evS, 1)
                        else:
                            vector.scalar_tensor_tensor(
                                dst,
                                psum[b][:, :],
                                2.0,
                                prev[:, t * COLS:(t + 1) * COLS],
                                op0=mybir.AluOpType.mult,
                                op1=mybir.AluOpType.subtract,
                            ).then_inc(evS, 1)

            @blk.scalar
            def _(scalar):
                for step in range(3):
                    cur = cur_arena[step]
                    for t in range(TILES):
                        scalar.wait_ge(evS, step * TILES + t + 1)
                        scalar.dma_start(
                            out=t_sl[step + 1][t * 128:(t + 1) * 128, :],
                            in_=cur[:, t * COLS:(t + 1) * COLS],
                        ).then_inc(stC[chunk_of_tile[t]], 16)

        # ---------------- phase 2
        with nc.Block() as blk2:

            @blk2.sync
            def _(sync):
                # theta/s/bias loads do not depend on phase-1 state
                sync.dma_start(out=ssb[:, :], in_=s_ext[:, :]).then_inc(p2L, 16)
                sync.dma_start(
                    out=thsb[:, :, :],
                    in_=th_ext[:, :].rearrange("(k f) o -> f k o", k=K1),
                ).then_inc(p2L, 16)
                sync.dma_start(out=bias_sb[:, :], in_=bias_ext[:, :]).then_inc(p2L, 16)
                for j in range(NCHUNK):
                    sync.wait_ge(stC[j], 16 * c_cnt[j] * 3)
                srcs = [xs_ext, t_sl[1], t_sl[2], t_sl[3]]
                # batch-parity double-buffered staging: arenas[bi%2] holds
                # the 4 transposed k-slices of batch bi (exactly 4*ROWPAD
                # cols), so batch bi+1 transposes overlap batch bi matmuls.
                for bi in range(B):
                    if bi >= 2:
                        sync.wait_ge(p2mm, (bi - 1) * TILES)
                    for k in range(K1):
                        dst = arenas[bi % 2][:, k * ROWPAD:k * ROWPAD + ROWPAD]
                        sync.dma_start_transpose(
                            dst, srcs[k][:, bi * F:(bi + 1) * F],
                        ).then_inc(p2T[bi % 2], 16)

            @blk2.tensor
            def _(tensor):
                tensor.wait_ge(onesS, 1)
                tensor.wait_ge(p2L, 48)
                for bi in range(B):
                    tensor.wait_ge(p2T[bi % 2], 16 * K1 * (bi // 2 + 1))
                    for t in range(TILES):
                        i = bi * TILES + t
                        pb = i % 4
                        if i >= 4:
                            tensor.wait_ge(p2ev, i - 3)
                        for k in range(K1):
                            src = arenas[bi % 2][:, k * ROWPAD + t * 128:
                                                 k * ROWPAD + (t + 1) * 128]
                            tensor.matmul(
                                ps2[pb][:, :],
                                src,
                                thsb[:, k, :],
                                start=(k == 0),
                                stop=False,
                            )
                        tensor.matmul(
                            ps2[pb][:, :],
                            ones1[:, :],
                            bias_sb[:, :],
                            start=False,
                            stop=True,
                        ).then_inc(p2mm, 1)

            @blk2.vector
            def _(vector):
                for bi in range(B):
                    for t in range(TILES):
                        i = bi * TILES + t
                        pb = i % 4
                        grp = bi * GRPB + t // 4
                        sl = t % 4
                        vector.wait_ge(p2mm, i + 1)
                        if sl == 0 and grp >= 2:
                            vector.wait_ge(p2st[grp % 2], 16 * (grp // 2))
                        dst = outsb[grp % 2][:, sl * F:(sl + 1) * F]
                        # same-engine in-order execution covers the
                        # write->read on dst; no sem round-trip needed
                        vector.tensor_scalar_mul(
                            dst, ps2[pb][:, 0:F], 1.0)
                        vector.scalar_tensor_tensor(
                            dst,
                            ps2[pb][:, F:2 * F],
                            ssb[:, (t * B + bi):(t * B + bi) + 1],
                            dst,
                            op0=mybir.AluOpType.mult,
                            op1=mybir.AluOpType.add,
                        ).then_inc(p2ev, 1)

            @blk2.scalar
            def _(scalar):
                for bi in range(B):
                    for t0 in range(0, TILES, 4):
                        gsz = min(4, TILES - t0)
                        grp = bi * GRPB + t0 // 4
                        i_last = bi * TILES + t0 + gsz - 1
                        scalar.wait_ge(p2ev, i_last + 1)
                        scalar.dma_start(
                            out=out_ext[bi * 128:(bi + 1) * 128,
                                        t0 * F:(t0 + gsz) * F],
                            in_=outsb[grp % 2][:, 0:gsz * F],
                        ).then_inc(p2st[grp % 2], 16)

        if DEBUG_DUMP:
            with nc.Block() as blk3:
                @blk3.sync
                def _(sync):
                    for _q in range(4):
                        sync.wait_ge(p2st[_q], 16)
                    base = 16
                    for i in range(PN // 128):
                        buf = msgs[i % 2][:, 0, :]
                        sync.dma_start(out=buf, in_=h_full[1][i * 128:(i + 1) * 128, :]).then_inc(idxS, 16)
                        sync.wait_ge(idxS, base + 16)
                        sync.dma_start(out=dbg1_ext[i * 128:(i + 1) * 128, :], in_=buf).then_inc(idxS, 16)
                        sync.wait_ge(idxS, base + 32)
                        base += 32
                    for i in range(ROWPAD // 128):
                        buf = msgs[i % 2][:, 0, :]
                        sync.dma_start(out=buf, in_=t_sl[3][i * 128:(i + 1) * 128, :]).then_inc(idxS, 16)
                        sync.wait_ge(idxS, base + 16)
                        sync.dma_start(out=dbg3_ext[i * 128:(i + 1) * 128, :], in_=buf).then_inc(idxS, 16)
                        sync.wait_ge(idxS, base + 32)
                        base += 32
                    for i in range(ROWPAD // 128):
                        buf = msgs[i % 2][:, 0, :]
                        sync.dma_start(out=buf, in_=t_sl[1][i * 128:(i + 1) * 128, :]).then_inc(idxS, 16)
                        sync.wait_ge(idxS, base + 16)
                        sync.dma_start(out=dbg0_ext[i * 128:(i + 1) * 128, :], in_=buf).then_inc(idxS, 16)
                        sync.wait_ge(idxS, base + 32)
                        base += 32

    nc.finalize()
    return nc


# ---------------------------------------------------------------- entry

_cache = {}


def _get_graph(N, B, F, K1, edge_index, edge_attr):
    key = (N, B, F, K1,
           hash(np.asarray(edge_index).tobytes()),
           hash(np.asarray(edge_attr).tobytes()))
    if key in _cache:
        return _cache[key]
    cfg, IDXW, WT_pm = _preprocess_edges(N, edge_index, edge_attr)
    nc = _build(cfg, B, F, K1)
    _cache.clear()
    _cache[key] = (cfg, IDXW, WT_pm, nc)
    return _cache[key]


def kernel(x, edge_index, edge_attr, s_local, Theta0, Theta1, bias):
    x = np.asarray(x)
    B, N, F = x.shape
    K1 = np.asarray(Theta0).shape[0]
    cfg, IDXW, WT_pm, nc = _get_graph(N, B, F, K1, edge_index, edge_attr)
    vals = _preprocess_values(cfg, x, s_local)
    SLICE, ROWPAD = cfg["SLICE"], cfg["ROWPAD"]
    COLS = vals["COLS"]

    th = np.concatenate([np.asarray(Theta0, np.float32),
                         np.asarray(Theta1, np.float32)], axis=2)
    th_b = np.ascontiguousarray(th).astype(ml_dtypes.bfloat16).reshape(K1 * F, 2 * F)
    bias2 = np.zeros((1, 2 * F), dtype=ml_dtypes.bfloat16)
    bias2[0, :F] = np.asarray(bias, np.float32).astype(ml_dtypes.bfloat16)

    in_maps = []
    for c in range(N_CORES):
        in_maps.append({
            "h0": vals["h0"],
            "idxw": np.ascontiguousarray(IDXW[c]),
            "wt": np.ascontiguousarray(WT_pm[c]),
            "x_slice": np.ascontiguousarray(vals["x_slice"][c]),
            "x_slice_pm": np.ascontiguousarray(vals["x_slice_pm"][c]),
            "s_pm": np.ascontiguousarray(vals["s_pm"][c]),
            "theta": th_b,
            "bias2": bias2,
        })

    trace = _maybe_install_ntff_hook()
    import tempfile
    tdir = tempfile.mkdtemp() if trace else None
    res = run_bass_kernel_spmd(nc, in_maps, core_ids=list(range(N_CORES)),
                               trace=trace, tmpdir=tdir)
    global last_exec_time_ns, last_trace_dir
    last_exec_time_ns = res.exec_time_ns
    last_trace_dir = tdir
    TILES = cfg["TILES"]
    out = np.empty((B, N, F), dtype=np.float32)
    for c in range(N_CORES):
        oc = res.results[c]["out"]  # [B*128, TILES*F] partition-major
        ocr = oc.reshape(B, 128, TILES, F).transpose(0, 2, 1, 3) \
                .reshape(B, ROWPAD, F)
        out[:, c * SLICE:(c + 1) * SLICE, :] = ocr[:, :SLICE]
    return out

